# revision 5
# baseline (speedup 1.0000x reference)
"""ONGNN (2-layer ordered-neuron GNN) on 8 Trainium2 NeuronCores — v2.

Same architecture as the baseline kernel (dst-node sharding, AllGather of
node features, indirect-DMA gather of source rows, one-hot-matmul segment
sum, node-parallel dense math), restructured to cut the SWDGE descriptor
load (the Q7 bottleneck) and overlap the collectives:

  - Edges are bucketed by (dst superwindow [512 nodes], src chunk-group)
    instead of (dst window [128], chunk): padding is only at bucket tails.
    Equalization pads gather row 0 (valid); slots beyond the per-bucket max
    real count hold idx=-1, which the gather ucode skips entirely.
  - One-hot tiles compare fp16 rel (dst offset within the superwindow,
    0..511; fp16 is exact for these) against a resident fp16 iota512, so a
    message tile may straddle windows; straddling tiles just get one extra
    matmul per extra window.
  - The node-feature table is AllGathered in 4 chunk-groups (pair-major row
    layout) so communication overlaps the dense phase that produces it and
    gathers start as soon as their chunk's group has arrived.
"""
import sys
import numpy as np

sys.path.insert(0, "/opt/trn_rl_repo")

import concourse.bass as bass
import concourse.bacc as bacc
import concourse.mybir as mybir
import concourse.tile as tile
from concourse import bass_utils

F = 128       # feature dim (IN_C == HID)
CH = 64       # CHUNK
OUT_C = 40
EPS = 1e-5
NCORES = 8

SH = 12500        # dst nodes per core
WPC = 98          # 128-node windows per core
SW = 4            # windows per superwindow
SHP = WPC * 128   # padded shard rows (12544)
N_SW = (WPC + SW - 1) // SW          # 25 superwindows
N_PAIRS = (N_SW + 1) // 2            # 13 pairs
# chunk-groups: pairs [0,4), [4,8), [8,12), [12,13)
GROUP_PAIRS = [(0, 4), (4, 8), (8, 12), (12, 13)]
NG = len(GROUP_PAIRS)
GROW_CORE = [4096, 4096, 4096, 256]       # rows per core per group
GROWS8 = [g * NCORES for g in GROW_CORE]  # 32768, 32768, 32768, 2048
GBASE = [0, 32768, 65536, 98304]
PAD_REL = 1000.0

FULL_CFG = dict(N=100000, E=1000000)


def _host_prep(x, edge_index, cfg):
    """Bucket edges by (core, superwindow, group); build idx/rel streams and
    the shared matmul span schedule (union over cores per tile)."""
    src = np.asarray(edge_index[0], dtype=np.int64)
    dst = np.asarray(edge_index[1], dtype=np.int64)

    k_src = src // SH
    r_src = src - k_src * SH
    g_src = r_src // 4096
    row = (np.asarray(GBASE, np.int64)[g_src]
           + k_src * np.asarray(GROW_CORE, np.int64)[g_src]
           + (r_src - g_src * 4096))
    idx_loc = row - np.asarray(GBASE, np.int64)[g_src]   # < 32768

    core = dst // SH
    dloc = dst - core * SH
    win = dloc >> 7
    sw = win >> 2
    rel = (dloc - sw * 512).astype(np.float32)           # 0..511

    bucket = ((core * N_SW + sw) * NG + g_src).astype(np.int64)
    order = np.lexsort((idx_loc, win, bucket))
    bcnt = np.bincount(bucket, minlength=NCORES * N_SW * NG) \
        .reshape(NCORES, N_SW, NG)
    maxreal = bcnt.max(axis=0)                           # [N_SW, NG]
    nt = -(-maxreal // 128)                              # tiles per bucket

    starts = np.zeros(NCORES * N_SW * NG + 1, np.int64)
    np.cumsum(bcnt.reshape(-1), out=starts[1:])

    # tile offsets per bucket (shared schedule)
    t0 = np.zeros((N_SW, NG), np.int64)
    acc = 0
    for s in range(N_SW):
        for g in range(NG):
            t0[s, g] = acc
            acc += nt[s, g]
    T = int(acc)

    idx16 = np.zeros((NCORES, T * 128), np.int16)
    relst = np.full((NCORES, T * 128), PAD_REL, np.float32)
    # spans: for each (sw, g, tile): set of local windows present (any core)
    span_sets = {}
    for s in range(N_SW):
        for g in range(NG):
            for t in range(int(nt[s, g])):
                span_sets[(s, g, t)] = set()

    for k in range(NCORES):
        for s in range(N_SW):
            for g in range(NG):
                b = (k * N_SW + s) * NG + g
                sel = order[starts[b]:starts[b + 1]]
                n = sel.size
                mr = int(maxreal[s, g])
                if mr == 0:
                    continue
                pos = t0[s, g] * 128
                idx16[k, pos:pos + n] = idx_loc[sel].astype(np.int16)
                # pads keep idx 0: gathered (valid row) but masked by rel
                relst[k, pos:pos + n] = rel[sel]
                wl = (rel[sel].astype(np.int64)) >> 7    # local window 0..3
                tl = np.arange(n) // 128
                for t, w in zip(tl, wl):
                    span_sets[(s, g, int(t))].add(int(w))

    # one matmul per (tile, window range): mm = (t_local, wlo, width, dummy)
    # width-1 mms first within each run so one-hot builds batch per width
    sched = []          # per sw: list of (g, t0, nt, mms_run)
    M = 0
    for s in range(N_SW):
        w_in_sw = min(SW, WPC - s * SW)
        runs = []
        covered = set()
        for g in range(NG):
            if nt[s, g] == 0:
                continue
            mm1, mm2 = [], []
            for t in range(int(nt[s, g])):
                W = span_sets[(s, g, t)] or {0}
                wlo, whi = min(W), max(W)
                if whi == wlo:
                    mm1.append((t, wlo, 1, False))
                    covered.add(wlo)
                elif whi - wlo == 1:
                    mm2.append((t, wlo, 2, False))
                    covered.update((wlo, wlo + 1))
                else:
                    mm2.append((t, wlo, 2, False))
                    w2 = min(wlo + 2, w_in_sw - 2)
                    mm2.append((t, w2, 2, False))
                    covered.update(range(wlo, wlo + 2))
                    covered.update(range(w2, w2 + 2))
            runs.append([g, int(t0[s, g]), int(nt[s, g]), mm1 + mm2])
        assert runs, f"superwindow {s} has no tiles"
        missing = set(range(w_in_sw)) - covered
        for w in sorted(missing):
            runs[0][3].insert(0, (0, w, 1, True))
        sched.append([tuple(r) for r in runs])
        M += sum(len(r[3]) for r in runs)

    # relmm: per matmul, the 128 messages' dst offsets within the matmul's
    # window range ([0, width*128), bf16-exact), or OOB
    relmm = np.full((NCORES, M, 128), PAD_REL, np.float32)
    mm_i = 0
    for s in range(N_SW):
        for g, gt0, gnt, mms_run in sched[s]:
            for t, wlo, width, dummy in mms_run:
                if not dummy:
                    for k in range(NCORES):
                        r = relst[k, (gt0 + t) * 128:(gt0 + t + 1) * 128] \
                            - wlo * 128
                        relmm[k, mm_i] = np.where(
                            (r >= 0) & (r < width * 128), r, PAD_REL)
                mm_i += 1
    assert mm_i == M

    idx16_dev = np.ascontiguousarray(
        idx16.reshape(NCORES, T * 8, 16).transpose(0, 2, 1))   # [NC,16,T*8]
    idx16_dev = np.tile(idx16_dev, (1, 8, 1))                  # [NC,128,T*8]
    rel_dev = np.ascontiguousarray(relmm.transpose(0, 2, 1))   # [NC,128,M]

    cnt = np.bincount(dst, minlength=cfg["N"]).astype(np.float32)
    inv = 1.0 / np.maximum(cnt, 1.0)
    inv_dev = np.ones((NCORES, SHP), np.float32)
    for k in range(NCORES):
        inv_dev[k, :SH] = inv[k * SH:(k + 1) * SH]
    inv_dev = np.ascontiguousarray(
        inv_dev.reshape(NCORES, WPC, 128).transpose(0, 2, 1))

    xT_dev = np.zeros((NCORES, F, SHP), np.float32)
    x = np.asarray(x, np.float32)
    for k in range(NCORES):
        xT_dev[k, :, :SH] = x[k * SH:(k + 1) * SH].T

    inv_fw = np.ones((NCORES, SHP), np.float32)
    for k in range(NCORES):
        inv_fw[k, :SH] = inv[k * SH:(k + 1) * SH]

    return dict(idx16=idx16_dev, rel=rel_dev, inv=inv_dev, inv_fw=inv_fw,
                xT=xT_dev, sched=sched, T=T, M=M)


def _affine_trivial(g, b):
    return bool(np.allclose(g, 1.0, atol=1e-7) and np.allclose(b, 0.0, atol=1e-7))


def build_nc(cfg, prep, flags, repeat=1):
    AOp = mybir.AluOpType
    AF = mybir.ActivationFunctionType
    dt = mybir.dt

    T = prep["T"]
    M = prep["M"]
    sched = prep["sched"]

    nc = bacc.Bacc("TRN2", target_bir_lowering=False, debug=False,
                   num_devices=NCORES)

    # ---- I/O ----
    xT_d = nc.dram_tensor("xT", [F, SHP], dt.bfloat16, kind="ExternalInput")
    win_d = nc.dram_tensor("w_in", [F, F], dt.bfloat16, kind="ExternalInput")
    idx_d = nc.dram_tensor("idx16", [128, T * 8], dt.int16, kind="ExternalInput")
    rel_d = nc.dram_tensor("rel", [128, M], dt.bfloat16, kind="ExternalInput")
    inv_d = nc.dram_tensor("inv", [128, WPC], dt.float32, kind="ExternalInput")
    invfw_d = nc.dram_tensor("invfw", [128, SHP], dt.float32,
                             kind="ExternalInput")
    tmw0_d = nc.dram_tensor("tmw0", [2 * F, CH], dt.bfloat16, kind="ExternalInput")
    tmw1_d = nc.dram_tensor("tmw1", [2 * F, CH], dt.bfloat16, kind="ExternalInput")
    wout_d = nc.dram_tensor("wout", [F, OUT_C], dt.bfloat16, kind="ExternalInput")
    iota_d = nc.dram_tensor("iota256", [128, 256], dt.bfloat16, kind="ExternalInput")
    ident_d = nc.dram_tensor("ident", [128, 128], dt.float32, kind="ExternalInput")
    out_d = nc.dram_tensor("out", [SHP, OUT_C], dt.float32, kind="ExternalOutput")
    gb_d = {}
    for nm in ("bin", "gin", "bein", "lng0", "lnb0", "lng1", "lnb1", "tmb0",
               "tmb1", "bout"):
        if not flags[nm + "_triv"]:
            width = {"tmb0": CH, "tmb1": CH, "bout": OUT_C}.get(nm, F)
            gb_d[nm] = nc.dram_tensor(nm, [128, width], dt.float32,
                                      kind="ExternalInput")

    bounce = [[nc.dram_tensor(f"bounce{l}g{g}", [GROW_CORE[g], F], dt.bfloat16)
               for g in range(NG)] for l in range(2)]
    hg = [[nc.dram_tensor(f"hfull{l}g{g}", [GROWS8[g], F], dt.bfloat16,
                          addr_space="Shared")
           for g in range(NG)] for l in range(2)]

    with tile.TileContext(nc) as tc:
        import contextlib
        ctx = contextlib.ExitStack()
        with ctx:
            res = ctx.enter_context(tc.tile_pool(name="res", bufs=1))
            gpool = ctx.enter_context(tc.tile_pool(name="gpool", bufs=4))
            ohpool = ctx.enter_context(tc.tile_pool(name="ohpool", bufs=3))
            stream = ctx.enter_context(tc.tile_pool(name="stream", bufs=2))
            tiny = ctx.enter_context(tc.tile_pool(name="tiny", bufs=3))
            small = ctx.enter_context(tc.tile_pool(name="small", bufs=3))
            psum = ctx.enter_context(tc.tile_pool(name="psum", bufs=3, space="PSUM"))
            pst = ctx.enter_context(tc.tile_pool(name="pst", bufs=2, space="PSUM"))

            # ---- residents / constants ----
            hT = res.tile([128, SHP], dt.bfloat16, tag="hT")
            h_own = res.tile([128, SHP], dt.bfloat16, tag="h_own")
            sigc = res.tile([128, WPC * CH], dt.bfloat16, tag="sigc")
            rel_t = res.tile([128, M], dt.bfloat16, tag="rel")
            idx_t = res.tile([128, T * 8], dt.int16, tag="idx")
            inv_t = res.tile([128, WPC], dt.float32, tag="inv")
            w_in = res.tile([F, F], dt.bfloat16, tag="w_in")
            tmw = [[res.tile([F, CH], dt.bfloat16, tag=f"tmw{l}{h}", name=f"tmw{l}{h}")
                    for h in range(2)] for l in range(2)]
            wout = res.tile([F, OUT_C], dt.bfloat16, tag="wout")
            iota_t = res.tile([128, 256], dt.bfloat16, tag="iota256")
            ident = res.tile([128, 128], dt.float32, tag="ident")
            scanmask = res.tile([128, 2 * SW * CH], dt.float32, tag="scanmask")
            gb_t = {}
            for nm, d in gb_d.items():
                gb_t[nm] = res.tile(list(d.shape), dt.float32, tag=nm, name=nm)
                nc.sync.dma_start(gb_t[nm][:], d.ap())

            nc.sync.dma_start(rel_t[:], rel_d.ap())
            nc.sync.dma_start(idx_t[:], idx_d.ap())
            nc.sync.dma_start(inv_t[:], inv_d.ap())
            nc.sync.dma_start(w_in[:], win_d.ap())
            for l, d in enumerate((tmw0_d, tmw1_d)):
                nc.sync.dma_start(tmw[l][0][:], d.ap()[0:F, :])
                nc.sync.dma_start(tmw[l][1][:], d.ap()[F:2 * F, :])
            nc.sync.dma_start(wout[:], wout_d.ap())
            nc.sync.dma_start(iota_t[:], iota_d.ap())
            nc.sync.dma_start(ident[:], ident_d.ap())
            eps_t = res.tile([128, 1], dt.float32, tag="eps")
            nc.vector.memset(eps_t[:], EPS)
            nc.vector.memset(scanmask[:], 1.0)
            nc.vector.memset(
                scanmask[:].rearrange("p (w c) -> p w c", c=CH)[:, :, 0:1], 0.0)

            # gather-slot hygiene: pad columns (idx=-1, skipped) read stale
            # SBUF; zero the slots once so the very first reads are finite.
            ntmax = max((r[2] for runs in sched for r in runs), default=1)
            for _ in range(4):
                z = gpool.tile([128, ntmax * 128], dt.bfloat16, tag="g",
                               name="gz")
                nc.vector.memset(z[:], 0.0)

            PW = 2 * SW
            n_pairs = N_PAIRS

            def ln_smalls(su, sq, nw):
                mean = small.tile([128, nw], dt.float32, tag="mean")
                nc.vector.tensor_scalar(mean[:], su, 1.0 / F, None, AOp.mult)
                t1 = small.tile([128, nw], dt.float32, tag="t1")
                nc.vector.tensor_tensor(t1[:], mean[:], su, AOp.mult)
                t2 = small.tile([128, nw], dt.float32, tag="t2")
                nc.vector.tensor_tensor(t2[:], sq, t1[:], AOp.subtract)
                srt = small.tile([128, nw], dt.float32, tag="srt")
                nc.scalar.activation(srt[:], t2[:], AF.Sqrt, bias=eps_t[:],
                                     scale=1.0 / F)
                rs = small.tile([128, nw], dt.float32, tag="rs")
                nc.vector.reciprocal(rs[:], srt[:])
                return mean, rs

            def apply_ln(dst_ap, u_ap, mean, rs, nw, gnm, bnm):
                u3 = u_ap.rearrange("p (w f) -> p w f", w=nw)
                d3 = dst_ap.rearrange("p (w f) -> p w f", w=nw)
                mb = mean[:].unsqueeze(2).broadcast_to([128, nw, 128])
                rb = rs[:].unsqueeze(2).broadcast_to([128, nw, 128])
                nc.vector.tensor_tensor(d3, u3, mb, AOp.subtract)
                nc.vector.tensor_tensor(d3, d3, rb, AOp.mult)
                if gnm is not None:
                    g3 = gb_t[gnm][:].unsqueeze(1).broadcast_to([128, nw, 128])
                    nc.vector.tensor_tensor(d3, d3, g3, AOp.mult)
                if bnm is not None:
                    b3 = gb_t[bnm][:].unsqueeze(1).broadcast_to([128, nw, 128])
                    nc.vector.tensor_tensor(d3, d3, b3, AOp.add)

            def pair_info(pr):
                sws = [sx for sx in (2 * pr, 2 * pr + 1) if sx < N_SW]
                pw0 = sws[0] * SW
                nwp = sum(min(SW, WPC - sx * SW) for sx in sws)
                return sws, pw0, nwp

            def group_of_pair(pr):
                for g, (a, b) in enumerate(GROUP_PAIRS):
                    if a <= pr < b:
                        return g
                raise AssertionError

            def pair_bounce_rows(pr, g):
                a, _ = GROUP_PAIRS[g]
                r0 = (2 * pr - 2 * a) * SW * 128
                return r0

            def _once():
                # ============ Phase A: h0 = LN(relu(x W + b)) ============
                for pr in range(n_pairs):
                    sws, pw0, nwp = pair_info(pr)
                    nwfp = nwp * 128
                    xt8 = stream.tile([128, PW * 128], dt.bfloat16, tag="xt4",
                                      name="xt8")
                    nc.sync.dma_start(xt8[:, :nwfp],
                                      xT_d.ap()[:, pw0 * 128:pw0 * 128 + nwfp])
                    r8 = stream.tile([128, PW * 128], dt.float32, tag="u4", name="r8")
                    for sw in sws:
                        w0 = sw * SW
                        nw = min(SW, WPC - w0)
                        nwf = nw * 128
                        off = (w0 - pw0) * 128
                        ps_z = psum.tile([128, SW * 128], dt.float32, tag="ps_acc")
                        for j in range(nw):
                            nc.tensor.matmul(ps_z[:, j * 128:(j + 1) * 128],
                                             xt8[:, off + j * 128:off + (j + 1) * 128],
                                             w_in[:], start=True, stop=True)
                        if "bin" in gb_t:
                            b3 = gb_t["bin"][:].unsqueeze(1).broadcast_to(
                                [128, nw, 128])
                            z3 = ps_z[:, :nwf].rearrange("p (w f) -> p w f", w=nw)
                            nc.vector.tensor_tensor(z3, z3, b3, AOp.add)
                        nc.scalar.activation(r8[:, off:off + nwf], ps_z[:, :nwf],
                                             AF.Relu)
                    su = small.tile([128, PW], dt.float32, tag="su")
                    nc.vector.tensor_reduce(
                        su[:, :nwp], r8[:, :nwfp].rearrange("p (w f) -> p w f", w=nwp),
                        mybir.AxisListType.X, AOp.add)
                    sqs = stream.tile([128, PW * 128], dt.float32, tag="e4", name="sqs")
                    nc.scalar.activation(sqs[:, :nwfp], r8[:, :nwfp], AF.Square)
                    sq = small.tile([128, PW], dt.float32, tag="sq")
                    nc.vector.tensor_reduce(
                        sq[:, :nwp],
                        sqs[:, :nwfp].rearrange("p (w f) -> p w f", w=nwp),
                        mybir.AxisListType.X, AOp.add)
                    mean, rs = ln_smalls(su[:, :nwp], sq[:, :nwp], nwp)
                    apply_ln(h_own[:, pw0 * 128:pw0 * 128 + nwfp], r8[:, :nwfp],
                             mean, rs, nwp,
                             "gin" if "gin" in gb_t else None,
                             "bein" if "bein" in gb_t else None)
                    g = group_of_pair(pr)
                    r0 = pair_bounce_rows(pr, g)
                    nc.sync.dma_start(
                        bounce[0][g].ap()[r0:r0 + nwfp, :]
                            .rearrange("(w p) f -> p w f", w=nwp),
                        h_own[:, pw0 * 128:pw0 * 128 + nwfp]
                            .rearrange("p (w f) -> p w f", w=nwp))
                    nc.sync.dma_start_transpose(
                        hT[:, pw0 * 128:pw0 * 128 + nwfp],
                        bounce[0][g].ap()[r0:r0 + nwfp, :])
                    if pr == GROUP_PAIRS[g][1] - 1:
                        nc.gpsimd.collective_compute(
                            "AllGather", AOp.bypass,
                            replica_groups=[list(range(NCORES))],
                            ins=[bounce[0][g].ap().opt()],
                            outs=[hg[0][g].ap().opt()])

                # ============ conv layers ============
                for l in range(2):
                    mm_base = 0
                    mm_sw0 = []
                    for sw in range(N_SW):
                        mm_sw0.append(mm_base)
                        mm_base += sum(len(r[3]) for r in sched[sw])
                    for pr in range(n_pairs):
                        sws, pw0, nwp = pair_info(pr)
                        nwfp = nwp * 128
                        mTf = stream.tile([128, PW * 128], dt.float32, tag="m4")
                        mt8 = tiny.tile([128, PW * 128], dt.bfloat16, tag="mt")
                        for sw in sws:
                            w0 = sw * SW
                            nw = min(SW, WPC - w0)
                            nwf = nw * 128
                            off = (w0 - pw0) * 128
                            runs = sched[sw]
                            ps_m = psum.tile([128, SW * 128], dt.float32,
                                             tag="ps_acc")
                            mm_i = mm_sw0[sw]
                            first = True
                            for ri, (g, gt0, gnt, mms_run) in enumerate(runs):
                                g_t = gpool.tile([128, gnt * 128], dt.bfloat16,
                                                 tag="g")
                                nc.gpsimd.dma_gather(
                                    g_t[:].rearrange("p (t f) -> p t f", t=gnt),
                                    hg[l][g].ap(),
                                    idx_t[:, gt0 * 8:(gt0 + gnt) * 8],
                                    gnt * 128, gnt * 128, F,
                                    single_packet=False)
                                nmm_r = len(mms_run)
                                n1 = sum(1 for mm in mms_run if mm[2] == 1)
                                n2 = nmm_r - n1
                                oh_t = ohpool.tile(
                                    [128, n1 * 128 + n2 * 256],
                                    dt.bfloat16, tag="oh")
                                if n1:
                                    oh3 = oh_t[:, :n1 * 128].rearrange(
                                        "p (m c) -> p m c", m=n1)
                                    iob = iota_t[:, :128].unsqueeze(1) \
                                        .broadcast_to([128, n1, 128])
                                    reb = rel_t[:, mm_i:mm_i + n1] \
                                        .unsqueeze(2) \
                                        .broadcast_to([128, n1, 128])
                                    nc.vector.tensor_tensor(oh3, iob, reb,
                                                            AOp.is_equal)
                                if n2:
                                    oh3 = oh_t[:, n1 * 128:].rearrange(
                                        "p (m c) -> p m c", m=n2)
                                    iob = iota_t[:].unsqueeze(1) \
                                        .broadcast_to([128, n2, 256])
                                    reb = rel_t[:, mm_i + n1:mm_i + nmm_r] \
                                        .unsqueeze(2) \
                                        .broadcast_to([128, n2, 256])
                                    nc.vector.tensor_tensor(oh3, iob, reb,
                                                            AOp.is_equal)
                                oh_off = 0
                                for j, (tl, wlo, width, dummy) in \
                                        enumerate(mms_run):
                                    is_last = (ri == len(runs) - 1
                                               and j == nmm_r - 1)
                                    wf = width * 128
                                    nc.tensor.matmul(
                                        ps_m[:, wlo * 128:wlo * 128 + wf],
                                        g_t[:, tl * 128:(tl + 1) * 128],
                                        oh_t[:, oh_off:oh_off + wf],
                                        start=first, stop=is_last)
                                    first = False
                                    oh_off += wf
                                mm_i += nmm_r
                            # mT (f32) into the pair tile (feature-major);
                            # ACT copy keeps the DVE SBUF port free for SWDGE
                            nc.scalar.activation(mTf[:, off:off + nwf],
                                                 ps_m[:, :nwf], AF.Copy)
                        # m~T (bf16, inv-scaled) for the transition matmul
                        ivf = tiny.tile([128, PW * 128], dt.float32, tag="ivf")
                        nc.sync.dma_start(
                            ivf[:, :nwfp],
                            invfw_d.ap()[:, pw0 * 128:pw0 * 128 + nwfp])
                        nc.vector.tensor_tensor(mt8[:, :nwfp], mTf[:, :nwfp],
                                                ivf[:, :nwfp], AOp.mult)
                        # m node-major (f32, inv-scaled): transpose mT
                        m4 = stream.tile([128, PW * 128], dt.float32, tag="m4n")
                        for half in range(0, nwp, SW):
                            nh = min(SW, nwp - half)
                            ps_t = pst.tile([128, SW * 128], dt.float32, tag="ps_t")
                            for j in range(nh):
                                nc.tensor.transpose(
                                    ps_t[:, j * 128:(j + 1) * 128],
                                    mTf[:, (half + j) * 128:(half + j + 1) * 128],
                                    ident[:])
                            m3 = m4[:, half * 128:(half + nh) * 128] \
                                .rearrange("p (w f) -> p w f", w=nh)
                            p3 = ps_t[:, :nh * 128] \
                                .rearrange("p (w f) -> p w f", w=nh)
                            iv = inv_t[:, pw0 + half:pw0 + half + nh] \
                                .unsqueeze(2).broadcast_to([128, nh, 128])
                            nc.vector.tensor_tensor(m3, p3, iv, AOp.mult)
                        # ---- dense phase over the whole pair ----
                        ps_tm = psum.tile([128, PW * CH], dt.float32, tag="ps_sm")
                        for j in range(nwp):
                            w = pw0 + j
                            nc.tensor.matmul(ps_tm[:, j * CH:(j + 1) * CH],
                                             hT[:, w * 128:(w + 1) * 128],
                                             tmw[l][0][:], start=True, stop=False)
                            nc.tensor.matmul(ps_tm[:, j * CH:(j + 1) * CH],
                                             mt8[:, j * 128:(j + 1) * 128],
                                             tmw[l][1][:], start=False, stop=True)
                        nwc = nwp * CH
                        if ("tmb0", "tmb1")[l] in gb_t:
                            tb = gb_t[("tmb0", "tmb1")[l]][:].unsqueeze(1) \
                                .broadcast_to([128, nwp, CH])
                            z3 = ps_tm[:, :nwc].rearrange("p (w c) -> p w c", w=nwp)
                            nc.vector.tensor_tensor(z3, z3, tb, AOp.add)
                        # softmax (no max-sub) + cumsum
                        e4 = stream.tile([128, PW * CH], dt.float32, tag="e4")
                        nc.scalar.activation(e4[:, :nwc], ps_tm[:, :nwc], AF.Exp)
                        s4 = small.tile([128, PW], dt.float32, tag="s4")
                        nc.vector.tensor_reduce(
                            s4[:, :nwp],
                            e4[:, :nwc].rearrange("p (w c) -> p w c", w=nwp),
                            mybir.AxisListType.X, AOp.add)
                        r4s = small.tile([128, PW], dt.float32, tag="r4s")
                        nc.vector.reciprocal(r4s[:, :nwp], s4[:, :nwp])
                        cs4 = stream.tile([128, PW * CH], dt.float32, tag="cs4")
                        nc.vector.tensor_tensor_scan(
                            cs4[:, :nwc], scanmask[:, :nwc], e4[:, :nwc],
                            0.0, AOp.mult, AOp.add)
                        # sig update
                        rb = r4s[:, :nwp].unsqueeze(2).broadcast_to([128, nwp, CH])
                        cs3 = cs4[:, :nwc].rearrange("p (w c) -> p w c", w=nwp)
                        sg_cols = sigc[:, pw0 * CH:pw0 * CH + nwc]
                        sg3 = sg_cols.rearrange("p (w c) -> p w c", w=nwp)
                        if l == 0:
                            nc.vector.tensor_tensor(sg3, cs3, rb, AOp.mult)
                            sig_src = sg_cols
                        else:
                            t4 = stream.tile([128, PW * CH], dt.float32, tag="t4")
                            t3 = t4[:, :nwc].rearrange("p (w c) -> p w c", w=nwp)
                            nc.vector.tensor_tensor(t3, cs3, rb, AOp.mult)
                            a4 = stream.tile([128, PW * CH], dt.float32, tag="a4")
                            nc.vector.tensor_tensor(a4[:, :nwc], sg_cols,
                                                    t4[:, :nwc], AOp.mult)
                            nc.vector.tensor_tensor(t4[:, :nwc], t4[:, :nwc],
                                                    a4[:, :nwc], AOp.subtract)
                            nc.vector.tensor_tensor(t4[:, :nwc], t4[:, :nwc],
                                                    sg_cols, AOp.add)
                            sig_src = t4[:, :nwc]
                        # mix u = m + sig*(h-m)
                        hcols = h_own[:, pw0 * 128:pw0 * 128 + nwfp]
                        u4 = stream.tile([128, PW * 128], dt.float32, tag="u4")
                        nc.vector.tensor_tensor(u4[:, :nwfp], hcols, m4[:, :nwfp],
                                                AOp.subtract)
                        src_b = sig_src.rearrange("p (w c) -> p w c", w=nwp) \
                            .unsqueeze(3).broadcast_to([128, nwp, CH, 2])
                        u4v = u4[:, :nwfp].rearrange("p (w c r) -> p w c r",
                                                     w=nwp, r=2)
                        nc.vector.tensor_tensor(u4v, u4v, src_b, AOp.mult)
                        nc.vector.tensor_tensor(u4[:, :nwfp], u4[:, :nwfp],
                                                m4[:, :nwfp], AOp.add)
                        # LN stats
                        su = small.tile([128, PW], dt.float32, tag="su")
                        nc.vector.tensor_reduce(
                            su[:, :nwp],
                            u4[:, :nwfp].rearrange("p (w f) -> p w f", w=nwp),
                            mybir.AxisListType.X, AOp.add)
                        sqs = stream.tile([128, PW * 128], dt.float32, tag="e4",
                                          name="sqs")
                        nc.scalar.activation(sqs[:, :nwfp], u4[:, :nwfp], AF.Square)
                        sq = small.tile([128, PW], dt.float32, tag="sq")
                        nc.vector.tensor_reduce(
                            sq[:, :nwp],
                            sqs[:, :nwfp].rearrange("p (w f) -> p w f", w=nwp),
                            mybir.AxisListType.X, AOp.add)
                        mean, rs = ln_smalls(su[:, :nwp], sq[:, :nwp], nwp)
                        gnm = ("lng0", "lng1")[l]
                        bnm = ("lnb0", "lnb1")[l]
                        if l == 0:
                            apply_ln(hcols, u4[:, :nwfp], mean, rs, nwp,
                                     gnm if gnm in gb_t else None,
                                     bnm if bnm in gb_t else None)
                            g = group_of_pair(pr)
                            r0 = pair_bounce_rows(pr, g)
                            nc.sync.dma_start(
                                bounce[1][g].ap()[r0:r0 + nwfp, :]
                                    .rearrange("(w p) f -> p w f", w=nwp),
                                hcols.rearrange("p (w f) -> p w f", w=nwp))
                            nc.sync.dma_start_transpose(
                                hT[:, pw0 * 128:pw0 * 128 + nwfp],
                                bounce[1][g].ap()[r0:r0 + nwfp, :])
                            if pr == GROUP_PAIRS[g][1] - 1:
                                nc.gpsimd.collective_compute(
                                    "AllGather", AOp.bypass,
                                    replica_groups=[list(range(NCORES))],
                                    ins=[bounce[1][g].ap().opt()],
                                    outs=[hg[1][g].ap().opt()])
                        else:
                            h2 = stream.tile([128, PW * 128], dt.float32, tag="hx",
                                             name="h2")
                            apply_ln(h2[:, :nwfp], u4[:, :nwfp], mean, rs, nwp,
                                     gnm if gnm in gb_t else None,
                                     bnm if bnm in gb_t else None)
                            ob = stream.tile([128, PW * OUT_C], dt.float32,
                                             tag="ob")
                            ps_o = psum.tile([128, PW * OUT_C], dt.float32,
                                             tag="ps_sm")
                            h2t8 = tiny.tile([128, PW * 128], dt.bfloat16,
                                             tag="h2t")
                            for half in range(0, nwp, SW):
                                nh = min(SW, nwp - half)
                                ps_t = pst.tile([128, SW * 128], dt.float32,
                                                tag="ps_t")
                                for j in range(nh):
                                    nc.tensor.transpose(
                                        ps_t[:, j * 128:(j + 1) * 128],
                                        h2[:, (half + j) * 128:(half + j + 1) * 128],
                                        ident[:])
                                nc.scalar.activation(
                                    h2t8[:, half * 128:(half + nh) * 128],
                                    ps_t[:, :nh * 128], AF.Copy)
                            for j in range(nwp):
                                nc.tensor.matmul(ps_o[:, j * OUT_C:(j + 1) * OUT_C],
                                                 h2t8[:, j * 128:(j + 1) * 128],
                                                 wout[:], start=True, stop=True)
                            nwo = nwp * OUT_C
                            if "bout" in gb_t:
                                bb = gb_t["bout"][:].unsqueeze(1).broadcast_to(
                                    [128, nwp, OUT_C])
                                o3 = ob[:, :nwo].rearrange("p (w o) -> p w o", w=nwp)
                                nc.vector.tensor_tensor(
                                    o3,
                                    ps_o[:, :nwo].rearrange("p (w o) -> p w o",
                                                            w=nwp),
                                    bb, AOp.add)
                            else:
                                nc.vector.tensor_copy(ob[:, :nwo], ps_o[:, :nwo])
                            nc.sync.dma_start(
                                out_d.ap()[pw0 * 128:pw0 * 128 + nwfp, :]
                                    .rearrange("(w p) o -> p w o", w=nwp),
                                ob[:, :nwo].rearrange("p (w o) -> p w o", w=nwp))

            for _rep in range(repeat):
                _once()

    nc.compile()
    return nc


_CACHE = {}


def _sched_key(prep):
    return tuple((g, t0, nt, tuple(mms))
                 for runs in prep["sched"] for g, t0, nt, mms in runs)


def _get_compiled(cfg, prep, flags, repeat=1):
    key = (_sched_key(prep), tuple(sorted(flags.items())), repeat)
    if key not in _CACHE:
        _CACHE[key] = build_nc(cfg, prep, flags, repeat=repeat)
    return _CACHE[key]


class PjrtRunner:
    """Persistent jitted shard_map executor for one compiled nc (8 cores)."""

    def __init__(self, nc):
        import jax
        from jax.experimental.shard_map import shard_map
        from jax.sharding import Mesh, PartitionSpec
        from concourse import bass2jax

        bass2jax.install_neuronx_cc_hook()
        self.nc = nc
        in_names, out_names, out_avals, zero_outs = [], [], [], []
        partition_name = (nc.partition_id_tensor.name
                          if nc.partition_id_tensor else None)
        for alloc in nc.m.functions[0].allocations:
            if not isinstance(alloc, mybir.MemoryLocationSet):
                continue
            name = alloc.memorylocations[0].name
            if alloc.kind == "ExternalInput":
                if name != partition_name:
                    in_names.append(name)
            elif alloc.kind == "ExternalOutput":
                out_names.append(name)
                aval = jax.core.ShapedArray(
                    tuple(alloc.tensor_shape), mybir.dt.np(alloc.dtype))
                out_avals.append(aval)
                zero_outs.append(np.zeros(alloc.tensor_shape,
                                          mybir.dt.np(alloc.dtype)))
        self.n_params = len(in_names)
        self.out_names = list(out_names)
        self.zero_outs = zero_outs
        all_in = in_names + out_names
        if partition_name is not None:
            all_in.append(partition_name)
        self.in_names_data = in_names

        def _body(*args):
            operands = list(args)
            if partition_name is not None:
                operands.append(bass2jax.partition_id_tensor())
            outs = bass2jax._bass_exec_p.bind(
                *operands,
                out_avals=tuple(out_avals),
                in_names=tuple(all_in),
                out_names=tuple(out_names),
                lowering_input_output_aliases=(),
                sim_require_finite=True,
                sim_require_nnan=True,
                nc=nc,
            )
            return tuple(outs)

        devices = jax.devices()[:NCORES]
        self.mesh = Mesh(np.asarray(devices), ("core",))
        n_out = len(out_names)
        donate = tuple(range(self.n_params, self.n_params + n_out))
        in_specs = (PartitionSpec("core"),) * (self.n_params + n_out)
        out_specs = (PartitionSpec("core"),) * n_out
        self.fn = jax.jit(
            shard_map(_body, mesh=self.mesh, in_specs=in_specs,
                      out_specs=out_specs, check_rep=False),
            donate_argnums=donate, keep_unused=True)

    def concat_inputs(self, in_maps):
        return [
            np.concatenate([np.asarray(in_maps[c][nm]) for c in range(NCORES)],
                           axis=0)
            for nm in self.in_names_data
        ]

    def zeros(self):
        return [np.zeros((NCORES * z.shape[0], *z.shape[1:]), z.dtype)
                for z in self.zero_outs]

    def __call__(self, concat_in, zeros):
        outs = self.fn(*concat_in, *zeros)
        return {nm: np.asarray(outs[i]) for i, nm in enumerate(self.out_names)}


_RUNNERS = {}


def get_runner(cfg, prep, flags, repeat=1):
    key = (_sched_key(prep), tuple(sorted(flags.items())), repeat)
    if key not in _RUNNERS:
        _RUNNERS[key] = PjrtRunner(_get_compiled(cfg, prep, flags, repeat=repeat))
    return _RUNNERS[key]


# Iterations per NEFF execution.  The axon client dispatch pipeline tops out
# at ~4 ms per execution; several iterations per execution make each dispatch
# device-bound so measured throughput reflects hardware time.
REPEAT = 4


def run(inputs, cfg):
    x = np.asarray(inputs["x"], np.float32)
    prep = _host_prep(x, np.asarray(inputs["edge_index"]), cfg)

    flags = make_flags(inputs)
    runner = get_runner(cfg, prep, flags, repeat=REPEAT)
    in_maps = make_in_maps(inputs, prep, flags)
    out = runner(runner.concat_inputs(in_maps), runner.zeros())["out"]
    out = out.reshape(NCORES, SHP, OUT_C)[:, :SH, :]
    return np.ascontiguousarray(out.reshape(NCORES * SH, OUT_C), dtype=np.float32)


def make_flags(inputs):
    return {
        "bin_triv": _affine_trivial(1.0, inputs["b_in"]),
        "gin_triv": _affine_trivial(inputs["g_in"], 0.0),
        "bein_triv": _affine_trivial(1.0, inputs["be_in"]),
        "lng0_triv": _affine_trivial(inputs["ln_g0"], 0.0),
        "lnb0_triv": _affine_trivial(1.0, inputs["ln_b0"]),
        "lng1_triv": _affine_trivial(inputs["ln_g1"], 0.0),
        "lnb1_triv": _affine_trivial(1.0, inputs["ln_b1"]),
        "tmb0_triv": _affine_trivial(1.0, inputs["tm_b0"]),
        "tmb1_triv": _affine_trivial(1.0, inputs["tm_b1"]),
        "bout_triv": _affine_trivial(1.0, inputs["b_out"]),
    }


def make_in_maps(inputs, prep, flags):
    import ml_dtypes
    bf16 = ml_dtypes.bfloat16

    def bc(v, width):
        return np.tile(np.asarray(v, np.float32).reshape(1, width), (128, 1))

    iota256 = np.tile(np.arange(256, dtype=np.float32)[None, :],
                      (128, 1)).astype(bf16)
    in_maps = []
    for k in range(NCORES):
        m = {
            "xT": prep["xT"][k].astype(bf16),
            "w_in": np.asarray(inputs["W_in"], np.float32).astype(bf16),
            "idx16": prep["idx16"][k],
            "rel": prep["rel"][k].astype(bf16),
            "inv": prep["inv"][k],
            "invfw": np.tile(prep["inv_fw"][k][None, :], (128, 1)),
            "tmw0": np.asarray(inputs["tm_W0"], np.float32).astype(bf16),
            "tmw1": np.asarray(inputs["tm_W1"], np.float32).astype(bf16),
            "wout": np.asarray(inputs["W_out"], np.float32).astype(bf16),
            "iota256": iota256,
            "ident": np.eye(128, dtype=np.float32),
        }
        if not flags["bin_triv"]:
            m["bin"] = bc(inputs["b_in"], F)
        if not flags["gin_triv"]:
            m["gin"] = bc(inputs["g_in"], F)
        if not flags["bein_triv"]:
            m["bein"] = bc(inputs["be_in"], F)
        for nm, src in (("lng0", "ln_g0"), ("lnb0", "ln_b0"),
                        ("lng1", "ln_g1"), ("lnb1", "ln_b1")):
            if not flags[nm + "_triv"]:
                m[nm] = bc(inputs[src], F)
        if not flags["tmb0_triv"]:
            m["tmb0"] = bc(inputs["tm_b0"], CH)
        if not flags["tmb1_triv"]:
            m["tmb1"] = bc(inputs["tm_b1"], CH)
        if not flags["bout_triv"]:
            m["bout"] = bc(inputs["b_out"], OUT_C)
        in_maps.append(m)
    return in_maps


def kernel(**inputs):
    return run(inputs, FULL_CFG)


# revision 6
# speedup vs baseline: 1.0071x; 1.0071x over previous
"""ONGNN (2-layer ordered-neuron GNN) on 8 Trainium2 NeuronCores — v2.

Same architecture as the baseline kernel (dst-node sharding, AllGather of
node features, indirect-DMA gather of source rows, one-hot-matmul segment
sum, node-parallel dense math), restructured to cut the SWDGE descriptor
load (the Q7 bottleneck) and overlap the collectives:

  - Edges are bucketed by (dst superwindow [512 nodes], src chunk-group)
    instead of (dst window [128], chunk): padding is only at bucket tails.
    Equalization pads gather row 0 (valid); slots beyond the per-bucket max
    real count hold idx=-1, which the gather ucode skips entirely.
  - One-hot tiles compare fp16 rel (dst offset within the superwindow,
    0..511; fp16 is exact for these) against a resident fp16 iota512, so a
    message tile may straddle windows; straddling tiles just get one extra
    matmul per extra window.
  - The node-feature table is AllGathered in 4 chunk-groups (pair-major row
    layout) so communication overlaps the dense phase that produces it and
    gathers start as soon as their chunk's group has arrived.
"""
import sys
import numpy as np

sys.path.insert(0, "/opt/trn_rl_repo")

import concourse.bass as bass
import concourse.bacc as bacc
import concourse.mybir as mybir
import concourse.tile as tile
from concourse import bass_utils

F = 128       # feature dim (IN_C == HID)
CH = 64       # CHUNK
OUT_C = 40
EPS = 1e-5
NCORES = 8

SH = 12500        # dst nodes per core
WPC = 98          # 128-node windows per core
SW = 4            # windows per superwindow
SHP = WPC * 128   # padded shard rows (12544)
N_SW = (WPC + SW - 1) // SW          # 25 superwindows
N_PAIRS = (N_SW + 1) // 2            # 13 pairs
# chunk-groups: pairs [0,4), [4,8), [8,12), [12,13)
GROUP_PAIRS = [(0, 4), (4, 8), (8, 12), (12, 13)]
NG = len(GROUP_PAIRS)
GROW_CORE = [4096, 4096, 4096, 256]       # rows per core per group
GROWS8 = [g * NCORES for g in GROW_CORE]  # 32768, 32768, 32768, 2048
GBASE = [0, 32768, 65536, 98304]
PAD_REL = 1000.0

FULL_CFG = dict(N=100000, E=1000000)


def _host_prep(x, edge_index, cfg):
    """Bucket edges by (core, superwindow, group); build idx/rel streams and
    the shared matmul span schedule (union over cores per tile)."""
    src = np.asarray(edge_index[0], dtype=np.int64)
    dst = np.asarray(edge_index[1], dtype=np.int64)

    k_src = src // SH
    r_src = src - k_src * SH
    g_src = r_src // 4096
    row = (np.asarray(GBASE, np.int64)[g_src]
           + k_src * np.asarray(GROW_CORE, np.int64)[g_src]
           + (r_src - g_src * 4096))
    idx_loc = row - np.asarray(GBASE, np.int64)[g_src]   # < 32768

    core = dst // SH
    dloc = dst - core * SH
    win = dloc >> 7
    sw = win >> 2
    rel = (dloc - sw * 512).astype(np.float32)           # 0..511

    bucket = ((core * N_SW + sw) * NG + g_src).astype(np.int64)
    order = np.lexsort((idx_loc, win, bucket))
    bcnt = np.bincount(bucket, minlength=NCORES * N_SW * NG) \
        .reshape(NCORES, N_SW, NG)
    maxreal = bcnt.max(axis=0)                           # [N_SW, NG]
    nt = -(-maxreal // 128)                              # tiles per bucket

    starts = np.zeros(NCORES * N_SW * NG + 1, np.int64)
    np.cumsum(bcnt.reshape(-1), out=starts[1:])

    # tile offsets per bucket (shared schedule)
    t0 = np.zeros((N_SW, NG), np.int64)
    acc = 0
    for s in range(N_SW):
        for g in range(NG):
            t0[s, g] = acc
            acc += nt[s, g]
    T = int(acc)

    idx16 = np.zeros((NCORES, T * 128), np.int16)
    relst = np.full((NCORES, T * 128), PAD_REL, np.float32)
    # spans: for each (sw, g, tile): set of local windows present (any core)
    span_sets = {}
    for s in range(N_SW):
        for g in range(NG):
            for t in range(int(nt[s, g])):
                span_sets[(s, g, t)] = set()

    for k in range(NCORES):
        for s in range(N_SW):
            for g in range(NG):
                b = (k * N_SW + s) * NG + g
                sel = order[starts[b]:starts[b + 1]]
                n = sel.size
                mr = int(maxreal[s, g])
                if mr == 0:
                    continue
                pos = t0[s, g] * 128
                idx16[k, pos:pos + n] = idx_loc[sel].astype(np.int16)
                # pads keep idx 0: gathered (valid row) but masked by rel
                relst[k, pos:pos + n] = rel[sel]
                wl = (rel[sel].astype(np.int64)) >> 7    # local window 0..3
                tl = np.arange(n) // 128
                for t, w in zip(tl, wl):
                    span_sets[(s, g, int(t))].add(int(w))

    # one matmul per (tile, window range): mm = (t_local, wlo, width, dummy)
    # width-1 mms first within each run so one-hot builds batch per width
    sched = []          # per sw: list of (g, t0, nt, mms_run)
    M = 0
    for s in range(N_SW):
        w_in_sw = min(SW, WPC - s * SW)
        runs = []
        covered = set()
        for g in range(NG):
            if nt[s, g] == 0:
                continue
            mm1, mm2 = [], []
            for t in range(int(nt[s, g])):
                W = span_sets[(s, g, t)] or {0}
                wlo, whi = min(W), max(W)
                if whi == wlo:
                    mm1.append((t, wlo, 1, False))
                    covered.add(wlo)
                elif whi - wlo == 1:
                    mm2.append((t, wlo, 2, False))
                    covered.update((wlo, wlo + 1))
                else:
                    mm2.append((t, wlo, 2, False))
                    w2 = min(wlo + 2, w_in_sw - 2)
                    mm2.append((t, w2, 2, False))
                    covered.update(range(wlo, wlo + 2))
                    covered.update(range(w2, w2 + 2))
            runs.append([g, int(t0[s, g]), int(nt[s, g]), mm1 + mm2])
        assert runs, f"superwindow {s} has no tiles"
        missing = set(range(w_in_sw)) - covered
        for w in sorted(missing):
            runs[0][3].insert(0, (0, w, 1, True))
        sched.append([tuple(r) for r in runs])
        M += sum(len(r[3]) for r in runs)

    # relmm: per matmul, the 128 messages' dst offsets within the matmul's
    # window range ([0, width*128), bf16-exact), or OOB
    relmm = np.full((NCORES, M, 128), PAD_REL, np.float32)
    mm_i = 0
    for s in range(N_SW):
        for g, gt0, gnt, mms_run in sched[s]:
            for t, wlo, width, dummy in mms_run:
                if not dummy:
                    for k in range(NCORES):
                        r = relst[k, (gt0 + t) * 128:(gt0 + t + 1) * 128] \
                            - wlo * 128
                        relmm[k, mm_i] = np.where(
                            (r >= 0) & (r < width * 128), r, PAD_REL)
                mm_i += 1
    assert mm_i == M

    idx16_dev = np.ascontiguousarray(
        idx16.reshape(NCORES, T * 8, 16).transpose(0, 2, 1))   # [NC,16,T*8]
    idx16_dev = np.tile(idx16_dev, (1, 8, 1))                  # [NC,128,T*8]
    rel_dev = np.ascontiguousarray(relmm.transpose(0, 2, 1))   # [NC,128,M]

    cnt = np.bincount(dst, minlength=cfg["N"]).astype(np.float32)
    inv = 1.0 / np.maximum(cnt, 1.0)
    inv_dev = np.ones((NCORES, SHP), np.float32)
    for k in range(NCORES):
        inv_dev[k, :SH] = inv[k * SH:(k + 1) * SH]
    inv_dev = np.ascontiguousarray(
        inv_dev.reshape(NCORES, WPC, 128).transpose(0, 2, 1))

    xT_dev = np.zeros((NCORES, F, SHP), np.float32)
    x = np.asarray(x, np.float32)
    for k in range(NCORES):
        xT_dev[k, :, :SH] = x[k * SH:(k + 1) * SH].T

    inv_fw = np.ones((NCORES, SHP), np.float32)
    for k in range(NCORES):
        inv_fw[k, :SH] = inv[k * SH:(k + 1) * SH]

    return dict(idx16=idx16_dev, rel=rel_dev, inv=inv_dev, inv_fw=inv_fw,
                xT=xT_dev, sched=sched, T=T, M=M)


def _affine_trivial(g, b):
    return bool(np.allclose(g, 1.0, atol=1e-7) and np.allclose(b, 0.0, atol=1e-7))


def build_nc(cfg, prep, flags, repeat=1):
    AOp = mybir.AluOpType
    AF = mybir.ActivationFunctionType
    dt = mybir.dt

    T = prep["T"]
    M = prep["M"]
    sched = prep["sched"]

    nc = bacc.Bacc("TRN2", target_bir_lowering=False, debug=False,
                   num_devices=NCORES)

    # ---- I/O ----
    xT_d = nc.dram_tensor("xT", [F, SHP], dt.bfloat16, kind="ExternalInput")
    win_d = nc.dram_tensor("w_in", [F, F], dt.bfloat16, kind="ExternalInput")
    idx_d = nc.dram_tensor("idx16", [128, T * 8], dt.int16, kind="ExternalInput")
    rel_d = nc.dram_tensor("rel", [128, M], dt.bfloat16, kind="ExternalInput")
    inv_d = nc.dram_tensor("inv", [128, WPC], dt.float32, kind="ExternalInput")
    invfw_d = nc.dram_tensor("invfw", [128, SHP], dt.float32,
                             kind="ExternalInput")
    tmw0_d = nc.dram_tensor("tmw0", [2 * F, CH], dt.bfloat16, kind="ExternalInput")
    tmw1_d = nc.dram_tensor("tmw1", [2 * F, CH], dt.bfloat16, kind="ExternalInput")
    wout_d = nc.dram_tensor("wout", [F, OUT_C], dt.bfloat16, kind="ExternalInput")
    iota_d = nc.dram_tensor("iota256", [128, 256], dt.bfloat16, kind="ExternalInput")
    ident_d = nc.dram_tensor("ident", [128, 128], dt.float32, kind="ExternalInput")
    out_d = nc.dram_tensor("out", [SHP, OUT_C], dt.float32, kind="ExternalOutput")
    gb_d = {}
    for nm in ("bin", "gin", "bein", "lng0", "lnb0", "lng1", "lnb1", "tmb0",
               "tmb1", "bout"):
        if not flags[nm + "_triv"]:
            width = {"tmb0": CH, "tmb1": CH, "bout": OUT_C}.get(nm, F)
            gb_d[nm] = nc.dram_tensor(nm, [128, width], dt.float32,
                                      kind="ExternalInput")

    bounce = [[nc.dram_tensor(f"bounce{l}g{g}", [GROW_CORE[g], F], dt.bfloat16)
               for g in range(NG)] for l in range(2)]
    hg = [[nc.dram_tensor(f"hfull{l}g{g}", [GROWS8[g], F], dt.bfloat16,
                          addr_space="Shared")
           for g in range(NG)] for l in range(2)]

    with tile.TileContext(nc) as tc:
        import contextlib
        ctx = contextlib.ExitStack()
        with ctx:
            res = ctx.enter_context(tc.tile_pool(name="res", bufs=1))
            gpool = ctx.enter_context(tc.tile_pool(name="gpool", bufs=3))
            ohpool = ctx.enter_context(tc.tile_pool(name="ohpool", bufs=2))
            stream = ctx.enter_context(tc.tile_pool(name="stream", bufs=2))
            tiny = ctx.enter_context(tc.tile_pool(name="tiny", bufs=3))
            small = ctx.enter_context(tc.tile_pool(name="small", bufs=3))
            psum = ctx.enter_context(tc.tile_pool(name="psum", bufs=3, space="PSUM"))
            pst = ctx.enter_context(tc.tile_pool(name="pst", bufs=2, space="PSUM"))

            # ---- residents / constants ----
            hT = res.tile([128, SHP], dt.bfloat16, tag="hT")
            h_own = res.tile([128, SHP], dt.bfloat16, tag="h_own")
            sigc = res.tile([128, WPC * CH], dt.bfloat16, tag="sigc")
            rel_t = res.tile([128, M], dt.bfloat16, tag="rel")
            idx_t = res.tile([128, T * 8], dt.int16, tag="idx")
            inv_t = res.tile([128, WPC], dt.float32, tag="inv")
            w_in = res.tile([F, F], dt.bfloat16, tag="w_in")
            tmw = [[res.tile([F, CH], dt.bfloat16, tag=f"tmw{l}{h}", name=f"tmw{l}{h}")
                    for h in range(2)] for l in range(2)]
            wout = res.tile([F, OUT_C], dt.bfloat16, tag="wout")
            iota_t = res.tile([128, 256], dt.bfloat16, tag="iota256")
            ident = res.tile([128, 128], dt.float32, tag="ident")
            scanmask = res.tile([128, 2 * SW * CH], dt.float32, tag="scanmask")
            gb_t = {}
            for nm, d in gb_d.items():
                gb_t[nm] = res.tile(list(d.shape), dt.float32, tag=nm, name=nm)
                nc.sync.dma_start(gb_t[nm][:], d.ap())

            nc.sync.dma_start(rel_t[:], rel_d.ap())
            nc.sync.dma_start(idx_t[:], idx_d.ap())
            nc.sync.dma_start(inv_t[:], inv_d.ap())
            nc.sync.dma_start(w_in[:], win_d.ap())
            for l, d in enumerate((tmw0_d, tmw1_d)):
                nc.sync.dma_start(tmw[l][0][:], d.ap()[0:F, :])
                nc.sync.dma_start(tmw[l][1][:], d.ap()[F:2 * F, :])
            nc.sync.dma_start(wout[:], wout_d.ap())
            nc.sync.dma_start(iota_t[:], iota_d.ap())
            nc.sync.dma_start(ident[:], ident_d.ap())
            eps_t = res.tile([128, 1], dt.float32, tag="eps")
            nc.vector.memset(eps_t[:], EPS)
            nc.vector.memset(scanmask[:], 1.0)
            nc.vector.memset(
                scanmask[:].rearrange("p (w c) -> p w c", c=CH)[:, :, 0:1], 0.0)

            # gather-slot hygiene: pad columns (idx=-1, skipped) read stale
            # SBUF; zero the slots once so the very first reads are finite.
            ntmax = max((r[2] for runs in sched for r in runs), default=1)
            for _ in range(3):
                z = gpool.tile([128, ntmax * 128], dt.bfloat16, tag="g",
                               name="gz")
                nc.vector.memset(z[:], 0.0)

            PW = 2 * SW
            n_pairs = N_PAIRS

            def ln_smalls(su, sq, nw):
                mean = small.tile([128, nw], dt.float32, tag="mean")
                nc.vector.tensor_scalar(mean[:], su, 1.0 / F, None, AOp.mult)
                t1 = small.tile([128, nw], dt.float32, tag="t1")
                nc.vector.tensor_tensor(t1[:], mean[:], su, AOp.mult)
                t2 = small.tile([128, nw], dt.float32, tag="t2")
                nc.vector.tensor_tensor(t2[:], sq, t1[:], AOp.subtract)
                srt = small.tile([128, nw], dt.float32, tag="srt")
                nc.scalar.activation(srt[:], t2[:], AF.Sqrt, bias=eps_t[:],
                                     scale=1.0 / F)
                rs = small.tile([128, nw], dt.float32, tag="rs")
                nc.vector.reciprocal(rs[:], srt[:])
                return mean, rs

            def apply_ln(dst_ap, u_ap, mean, rs, nw, gnm, bnm):
                u3 = u_ap.rearrange("p (w f) -> p w f", w=nw)
                d3 = dst_ap.rearrange("p (w f) -> p w f", w=nw)
                mb = mean[:].unsqueeze(2).broadcast_to([128, nw, 128])
                rb = rs[:].unsqueeze(2).broadcast_to([128, nw, 128])
                nc.vector.tensor_tensor(d3, u3, mb, AOp.subtract)
                nc.vector.tensor_tensor(d3, d3, rb, AOp.mult)
                if gnm is not None:
                    g3 = gb_t[gnm][:].unsqueeze(1).broadcast_to([128, nw, 128])
                    nc.vector.tensor_tensor(d3, d3, g3, AOp.mult)
                if bnm is not None:
                    b3 = gb_t[bnm][:].unsqueeze(1).broadcast_to([128, nw, 128])
                    nc.vector.tensor_tensor(d3, d3, b3, AOp.add)

            def pair_info(pr):
                sws = [sx for sx in (2 * pr, 2 * pr + 1) if sx < N_SW]
                pw0 = sws[0] * SW
                nwp = sum(min(SW, WPC - sx * SW) for sx in sws)
                return sws, pw0, nwp

            def group_of_pair(pr):
                for g, (a, b) in enumerate(GROUP_PAIRS):
                    if a <= pr < b:
                        return g
                raise AssertionError

            def pair_bounce_rows(pr, g):
                a, _ = GROUP_PAIRS[g]
                r0 = (2 * pr - 2 * a) * SW * 128
                return r0

            def _once():
                # ============ Phase A: h0 = LN(relu(x W + b)) ============
                for pr in range(n_pairs):
                    sws, pw0, nwp = pair_info(pr)
                    nwfp = nwp * 128
                    xt8 = stream.tile([128, PW * 128], dt.bfloat16, tag="xt4",
                                      name="xt8")
                    nc.sync.dma_start(xt8[:, :nwfp],
                                      xT_d.ap()[:, pw0 * 128:pw0 * 128 + nwfp])
                    r8 = stream.tile([128, PW * 128], dt.float32, tag="u4", name="r8")
                    for sw in sws:
                        w0 = sw * SW
                        nw = min(SW, WPC - w0)
                        nwf = nw * 128
                        off = (w0 - pw0) * 128
                        ps_z = psum.tile([128, SW * 128], dt.float32, tag="ps_acc")
                        for j in range(nw):
                            nc.tensor.matmul(ps_z[:, j * 128:(j + 1) * 128],
                                             xt8[:, off + j * 128:off + (j + 1) * 128],
                                             w_in[:], start=True, stop=True)
                        if "bin" in gb_t:
                            b3 = gb_t["bin"][:].unsqueeze(1).broadcast_to(
                                [128, nw, 128])
                            z3 = ps_z[:, :nwf].rearrange("p (w f) -> p w f", w=nw)
                            nc.vector.tensor_tensor(z3, z3, b3, AOp.add)
                        nc.scalar.activation(r8[:, off:off + nwf], ps_z[:, :nwf],
                                             AF.Relu)
                    su = small.tile([128, PW], dt.float32, tag="su")
                    nc.vector.tensor_reduce(
                        su[:, :nwp], r8[:, :nwfp].rearrange("p (w f) -> p w f", w=nwp),
                        mybir.AxisListType.X, AOp.add)
                    sqs = stream.tile([128, PW * 128], dt.float32, tag="e4", name="sqs")
                    nc.scalar.activation(sqs[:, :nwfp], r8[:, :nwfp], AF.Square)
                    sq = small.tile([128, PW], dt.float32, tag="sq")
                    nc.vector.tensor_reduce(
                        sq[:, :nwp],
                        sqs[:, :nwfp].rearrange("p (w f) -> p w f", w=nwp),
                        mybir.AxisListType.X, AOp.add)
                    mean, rs = ln_smalls(su[:, :nwp], sq[:, :nwp], nwp)
                    apply_ln(h_own[:, pw0 * 128:pw0 * 128 + nwfp], r8[:, :nwfp],
                             mean, rs, nwp,
                             "gin" if "gin" in gb_t else None,
                             "bein" if "bein" in gb_t else None)
                    g = group_of_pair(pr)
                    r0 = pair_bounce_rows(pr, g)
                    nc.sync.dma_start(
                        bounce[0][g].ap()[r0:r0 + nwfp, :]
                            .rearrange("(w p) f -> p w f", w=nwp),
                        h_own[:, pw0 * 128:pw0 * 128 + nwfp]
                            .rearrange("p (w f) -> p w f", w=nwp))
                    nc.sync.dma_start_transpose(
                        hT[:, pw0 * 128:pw0 * 128 + nwfp],
                        bounce[0][g].ap()[r0:r0 + nwfp, :])
                    if pr == GROUP_PAIRS[g][1] - 1:
                        nc.gpsimd.collective_compute(
                            "AllGather", AOp.bypass,
                            replica_groups=[list(range(NCORES))],
                            ins=[bounce[0][g].ap().opt()],
                            outs=[hg[0][g].ap().opt()])

                # ============ conv layers ============
                for l in range(2):
                    mm_base = 0
                    mm_sw0 = []
                    for sw in range(N_SW):
                        mm_sw0.append(mm_base)
                        mm_base += sum(len(r[3]) for r in sched[sw])
                    for pr in range(n_pairs):
                        sws, pw0, nwp = pair_info(pr)
                        nwfp = nwp * 128
                        mTf = stream.tile([128, PW * 128], dt.float32, tag="m4")
                        mt8 = tiny.tile([128, PW * 128], dt.bfloat16, tag="mt")
                        for sw in sws:
                            w0 = sw * SW
                            nw = min(SW, WPC - w0)
                            nwf = nw * 128
                            off = (w0 - pw0) * 128
                            runs = sched[sw]
                            ps_m = psum.tile([128, SW * 128], dt.float32,
                                             tag="ps_acc")
                            mm_i = mm_sw0[sw]
                            first = True
                            for ri, (g, gt0, gnt, mms_run) in enumerate(runs):
                                g_t = gpool.tile([128, gnt * 128], dt.bfloat16,
                                                 tag="g")
                                nc.gpsimd.dma_gather(
                                    g_t[:].rearrange("p (t f) -> p t f", t=gnt),
                                    hg[l][g].ap(),
                                    idx_t[:, gt0 * 8:(gt0 + gnt) * 8],
                                    gnt * 128, gnt * 128, F,
                                    single_packet=False)
                                nmm_r = len(mms_run)
                                n1 = sum(1 for mm in mms_run if mm[2] == 1)
                                n2 = nmm_r - n1
                                oh_t = ohpool.tile(
                                    [128, n1 * 128 + n2 * 256],
                                    dt.bfloat16, tag="oh")
                                if n1:
                                    oh3 = oh_t[:, :n1 * 128].rearrange(
                                        "p (m c) -> p m c", m=n1)
                                    iob = iota_t[:, :128].unsqueeze(1) \
                                        .broadcast_to([128, n1, 128])
                                    reb = rel_t[:, mm_i:mm_i + n1] \
                                        .unsqueeze(2) \
                                        .broadcast_to([128, n1, 128])
                                    nc.vector.tensor_tensor(oh3, iob, reb,
                                                            AOp.is_equal)
                                if n2:
                                    oh3 = oh_t[:, n1 * 128:].rearrange(
                                        "p (m c) -> p m c", m=n2)
                                    iob = iota_t[:].unsqueeze(1) \
                                        .broadcast_to([128, n2, 256])
                                    reb = rel_t[:, mm_i + n1:mm_i + nmm_r] \
                                        .unsqueeze(2) \
                                        .broadcast_to([128, n2, 256])
                                    nc.vector.tensor_tensor(oh3, iob, reb,
                                                            AOp.is_equal)
                                oh_off = 0
                                for j, (tl, wlo, width, dummy) in \
                                        enumerate(mms_run):
                                    is_last = (ri == len(runs) - 1
                                               and j == nmm_r - 1)
                                    wf = width * 128
                                    nc.tensor.matmul(
                                        ps_m[:, wlo * 128:wlo * 128 + wf],
                                        g_t[:, tl * 128:(tl + 1) * 128],
                                        oh_t[:, oh_off:oh_off + wf],
                                        start=first, stop=is_last)
                                    first = False
                                    oh_off += wf
                                mm_i += nmm_r
                            # mT (f32) into the pair tile (feature-major);
                            # ACT copy keeps the DVE SBUF port free for SWDGE
                            nc.scalar.activation(mTf[:, off:off + nwf],
                                                 ps_m[:, :nwf], AF.Copy)
                        # m~T (bf16, inv-scaled) for the transition matmul
                        ivf = tiny.tile([128, PW * 128], dt.float32, tag="ivf")
                        nc.sync.dma_start(
                            ivf[:, :nwfp],
                            invfw_d.ap()[:, pw0 * 128:pw0 * 128 + nwfp])
                        nc.vector.tensor_tensor(mt8[:, :nwfp], mTf[:, :nwfp],
                                                ivf[:, :nwfp], AOp.mult)
                        # m node-major (f32, inv-scaled): transpose mT
                        m4 = stream.tile([128, PW * 128], dt.float32, tag="m4n")
                        for half in range(0, nwp, SW):
                            nh = min(SW, nwp - half)
                            ps_t = pst.tile([128, SW * 128], dt.float32, tag="ps_t")
                            for j in range(nh):
                                nc.tensor.transpose(
                                    ps_t[:, j * 128:(j + 1) * 128],
                                    mTf[:, (half + j) * 128:(half + j + 1) * 128],
                                    ident[:])
                            m3 = m4[:, half * 128:(half + nh) * 128] \
                                .rearrange("p (w f) -> p w f", w=nh)
                            p3 = ps_t[:, :nh * 128] \
                                .rearrange("p (w f) -> p w f", w=nh)
                            iv = inv_t[:, pw0 + half:pw0 + half + nh] \
                                .unsqueeze(2).broadcast_to([128, nh, 128])
                            nc.vector.tensor_tensor(m3, p3, iv, AOp.mult)
                        # ---- dense phase over the whole pair ----
                        ps_tm = psum.tile([128, PW * CH], dt.float32, tag="ps_sm")
                        for j in range(nwp):
                            w = pw0 + j
                            nc.tensor.matmul(ps_tm[:, j * CH:(j + 1) * CH],
                                             hT[:, w * 128:(w + 1) * 128],
                                             tmw[l][0][:], start=True, stop=False)
                            nc.tensor.matmul(ps_tm[:, j * CH:(j + 1) * CH],
                                             mt8[:, j * 128:(j + 1) * 128],
                                             tmw[l][1][:], start=False, stop=True)
                        nwc = nwp * CH
                        if ("tmb0", "tmb1")[l] in gb_t:
                            tb = gb_t[("tmb0", "tmb1")[l]][:].unsqueeze(1) \
                                .broadcast_to([128, nwp, CH])
                            z3 = ps_tm[:, :nwc].rearrange("p (w c) -> p w c", w=nwp)
                            nc.vector.tensor_tensor(z3, z3, tb, AOp.add)
                        # softmax (no max-sub) + cumsum
                        e4 = stream.tile([128, PW * CH], dt.float32, tag="e4")
                        nc.scalar.activation(e4[:, :nwc], ps_tm[:, :nwc], AF.Exp)
                        s4 = small.tile([128, PW], dt.float32, tag="s4")
                        nc.vector.tensor_reduce(
                            s4[:, :nwp],
                            e4[:, :nwc].rearrange("p (w c) -> p w c", w=nwp),
                            mybir.AxisListType.X, AOp.add)
                        r4s = small.tile([128, PW], dt.float32, tag="r4s")
                        nc.vector.reciprocal(r4s[:, :nwp], s4[:, :nwp])
                        cs4 = stream.tile([128, PW * CH], dt.float32, tag="cs4")
                        nc.vector.tensor_tensor_scan(
                            cs4[:, :nwc], scanmask[:, :nwc], e4[:, :nwc],
                            0.0, AOp.mult, AOp.add)
                        # sig update
                        rb = r4s[:, :nwp].unsqueeze(2).broadcast_to([128, nwp, CH])
                        cs3 = cs4[:, :nwc].rearrange("p (w c) -> p w c", w=nwp)
                        sg_cols = sigc[:, pw0 * CH:pw0 * CH + nwc]
                        sg3 = sg_cols.rearrange("p (w c) -> p w c", w=nwp)
                        if l == 0:
                            nc.vector.tensor_tensor(sg3, cs3, rb, AOp.mult)
                            sig_src = sg_cols
                        else:
                            t4 = stream.tile([128, PW * CH], dt.float32, tag="t4")
                            t3 = t4[:, :nwc].rearrange("p (w c) -> p w c", w=nwp)
                            nc.vector.tensor_tensor(t3, cs3, rb, AOp.mult)
                            a4 = stream.tile([128, PW * CH], dt.float32, tag="a4")
                            nc.vector.tensor_tensor(a4[:, :nwc], sg_cols,
                                                    t4[:, :nwc], AOp.mult)
                            nc.vector.tensor_tensor(t4[:, :nwc], t4[:, :nwc],
                                                    a4[:, :nwc], AOp.subtract)
                            nc.vector.tensor_tensor(t4[:, :nwc], t4[:, :nwc],
                                                    sg_cols, AOp.add)
                            sig_src = t4[:, :nwc]
                        # mix u = m + sig*(h-m)
                        hcols = h_own[:, pw0 * 128:pw0 * 128 + nwfp]
                        u4 = stream.tile([128, PW * 128], dt.float32, tag="u4")
                        nc.vector.tensor_tensor(u4[:, :nwfp], hcols, m4[:, :nwfp],
                                                AOp.subtract)
                        src_b = sig_src.rearrange("p (w c) -> p w c", w=nwp) \
                            .unsqueeze(3).broadcast_to([128, nwp, CH, 2])
                        u4v = u4[:, :nwfp].rearrange("p (w c r) -> p w c r",
                                                     w=nwp, r=2)
                        nc.vector.tensor_tensor(u4v, u4v, src_b, AOp.mult)
                        nc.vector.tensor_tensor(u4[:, :nwfp], u4[:, :nwfp],
                                                m4[:, :nwfp], AOp.add)
                        # LN stats
                        su = small.tile([128, PW], dt.float32, tag="su")
                        nc.vector.tensor_reduce(
                            su[:, :nwp],
                            u4[:, :nwfp].rearrange("p (w f) -> p w f", w=nwp),
                            mybir.AxisListType.X, AOp.add)
                        sqs = stream.tile([128, PW * 128], dt.float32, tag="e4",
                                          name="sqs")
                        nc.scalar.activation(sqs[:, :nwfp], u4[:, :nwfp], AF.Square)
                        sq = small.tile([128, PW], dt.float32, tag="sq")
                        nc.vector.tensor_reduce(
                            sq[:, :nwp],
                            sqs[:, :nwfp].rearrange("p (w f) -> p w f", w=nwp),
                            mybir.AxisListType.X, AOp.add)
                        mean, rs = ln_smalls(su[:, :nwp], sq[:, :nwp], nwp)
                        gnm = ("lng0", "lng1")[l]
                        bnm = ("lnb0", "lnb1")[l]
                        if l == 0:
                            apply_ln(hcols, u4[:, :nwfp], mean, rs, nwp,
                                     gnm if gnm in gb_t else None,
                                     bnm if bnm in gb_t else None)
                            g = group_of_pair(pr)
                            r0 = pair_bounce_rows(pr, g)
                            nc.sync.dma_start(
                                bounce[1][g].ap()[r0:r0 + nwfp, :]
                                    .rearrange("(w p) f -> p w f", w=nwp),
                                hcols.rearrange("p (w f) -> p w f", w=nwp))
                            nc.sync.dma_start_transpose(
                                hT[:, pw0 * 128:pw0 * 128 + nwfp],
                                bounce[1][g].ap()[r0:r0 + nwfp, :])
                            if pr == GROUP_PAIRS[g][1] - 1:
                                nc.gpsimd.collective_compute(
                                    "AllGather", AOp.bypass,
                                    replica_groups=[list(range(NCORES))],
                                    ins=[bounce[1][g].ap().opt()],
                                    outs=[hg[1][g].ap().opt()])
                        else:
                            h2 = stream.tile([128, PW * 128], dt.float32, tag="hx",
                                             name="h2")
                            apply_ln(h2[:, :nwfp], u4[:, :nwfp], mean, rs, nwp,
                                     gnm if gnm in gb_t else None,
                                     bnm if bnm in gb_t else None)
                            ob = stream.tile([128, PW * OUT_C], dt.float32,
                                             tag="ob")
                            ps_o = psum.tile([128, PW * OUT_C], dt.float32,
                                             tag="ps_sm")
                            h2t8 = tiny.tile([128, PW * 128], dt.bfloat16,
                                             tag="h2t")
                            for half in range(0, nwp, SW):
                                nh = min(SW, nwp - half)
                                ps_t = pst.tile([128, SW * 128], dt.float32,
                                                tag="ps_t")
                                for j in range(nh):
                                    nc.tensor.transpose(
                                        ps_t[:, j * 128:(j + 1) * 128],
                                        h2[:, (half + j) * 128:(half + j + 1) * 128],
                                        ident[:])
                                nc.scalar.activation(
                                    h2t8[:, half * 128:(half + nh) * 128],
                                    ps_t[:, :nh * 128], AF.Copy)
                            for j in range(nwp):
                                nc.tensor.matmul(ps_o[:, j * OUT_C:(j + 1) * OUT_C],
                                                 h2t8[:, j * 128:(j + 1) * 128],
                                                 wout[:], start=True, stop=True)
                            nwo = nwp * OUT_C
                            if "bout" in gb_t:
                                bb = gb_t["bout"][:].unsqueeze(1).broadcast_to(
                                    [128, nwp, OUT_C])
                                o3 = ob[:, :nwo].rearrange("p (w o) -> p w o", w=nwp)
                                nc.vector.tensor_tensor(
                                    o3,
                                    ps_o[:, :nwo].rearrange("p (w o) -> p w o",
                                                            w=nwp),
                                    bb, AOp.add)
                            else:
                                nc.vector.tensor_copy(ob[:, :nwo], ps_o[:, :nwo])
                            nc.sync.dma_start(
                                out_d.ap()[pw0 * 128:pw0 * 128 + nwfp, :]
                                    .rearrange("(w p) o -> p w o", w=nwp),
                                ob[:, :nwo].rearrange("p (w o) -> p w o", w=nwp))

            for _rep in range(repeat):
                _once()

    nc.compile()
    return nc


_CACHE = {}


def _sched_key(prep):
    return tuple((g, t0, nt, tuple(mms))
                 for runs in prep["sched"] for g, t0, nt, mms in runs)


def _get_compiled(cfg, prep, flags, repeat=1):
    key = (_sched_key(prep), tuple(sorted(flags.items())), repeat)
    if key not in _CACHE:
        _CACHE[key] = build_nc(cfg, prep, flags, repeat=repeat)
    return _CACHE[key]


class PjrtRunner:
    """Persistent jitted shard_map executor for one compiled nc (8 cores)."""

    def __init__(self, nc):
        import jax
        from jax.experimental.shard_map import shard_map
        from jax.sharding import Mesh, PartitionSpec
        from concourse import bass2jax

        bass2jax.install_neuronx_cc_hook()
        self.nc = nc
        in_names, out_names, out_avals, zero_outs = [], [], [], []
        partition_name = (nc.partition_id_tensor.name
                          if nc.partition_id_tensor else None)
        for alloc in nc.m.functions[0].allocations:
            if not isinstance(alloc, mybir.MemoryLocationSet):
                continue
            name = alloc.memorylocations[0].name
            if alloc.kind == "ExternalInput":
                if name != partition_name:
                    in_names.append(name)
            elif alloc.kind == "ExternalOutput":
                out_names.append(name)
                aval = jax.core.ShapedArray(
                    tuple(alloc.tensor_shape), mybir.dt.np(alloc.dtype))
                out_avals.append(aval)
                zero_outs.append(np.zeros(alloc.tensor_shape,
                                          mybir.dt.np(alloc.dtype)))
        self.n_params = len(in_names)
        self.out_names = list(out_names)
        self.zero_outs = zero_outs
        all_in = in_names + out_names
        if partition_name is not None:
            all_in.append(partition_name)
        self.in_names_data = in_names

        def _body(*args):
            operands = list(args)
            if partition_name is not None:
                operands.append(bass2jax.partition_id_tensor())
            outs = bass2jax._bass_exec_p.bind(
                *operands,
                out_avals=tuple(out_avals),
                in_names=tuple(all_in),
                out_names=tuple(out_names),
                lowering_input_output_aliases=(),
                sim_require_finite=True,
                sim_require_nnan=True,
                nc=nc,
            )
            return tuple(outs)

        devices = jax.devices()[:NCORES]
        self.mesh = Mesh(np.asarray(devices), ("core",))
        n_out = len(out_names)
        donate = tuple(range(self.n_params, self.n_params + n_out))
        in_specs = (PartitionSpec("core"),) * (self.n_params + n_out)
        out_specs = (PartitionSpec("core"),) * n_out
        self.fn = jax.jit(
            shard_map(_body, mesh=self.mesh, in_specs=in_specs,
                      out_specs=out_specs, check_rep=False),
            donate_argnums=donate, keep_unused=True)

    def concat_inputs(self, in_maps):
        return [
            np.concatenate([np.asarray(in_maps[c][nm]) for c in range(NCORES)],
                           axis=0)
            for nm in self.in_names_data
        ]

    def zeros(self):
        return [np.zeros((NCORES * z.shape[0], *z.shape[1:]), z.dtype)
                for z in self.zero_outs]

    def __call__(self, concat_in, zeros):
        outs = self.fn(*concat_in, *zeros)
        return {nm: np.asarray(outs[i]) for i, nm in enumerate(self.out_names)}


_RUNNERS = {}


def get_runner(cfg, prep, flags, repeat=1):
    key = (_sched_key(prep), tuple(sorted(flags.items())), repeat)
    if key not in _RUNNERS:
        _RUNNERS[key] = PjrtRunner(_get_compiled(cfg, prep, flags, repeat=repeat))
    return _RUNNERS[key]


# Iterations per NEFF execution.  The axon client dispatch pipeline tops out
# at ~4 ms per execution; several iterations per execution make each dispatch
# device-bound so measured throughput reflects hardware time.
REPEAT = 4


def run(inputs, cfg):
    x = np.asarray(inputs["x"], np.float32)
    prep = _host_prep(x, np.asarray(inputs["edge_index"]), cfg)

    flags = make_flags(inputs)
    runner = get_runner(cfg, prep, flags, repeat=REPEAT)
    in_maps = make_in_maps(inputs, prep, flags)
    out = runner(runner.concat_inputs(in_maps), runner.zeros())["out"]
    out = out.reshape(NCORES, SHP, OUT_C)[:, :SH, :]
    return np.ascontiguousarray(out.reshape(NCORES * SH, OUT_C), dtype=np.float32)


def make_flags(inputs):
    return {
        "bin_triv": _affine_trivial(1.0, inputs["b_in"]),
        "gin_triv": _affine_trivial(inputs["g_in"], 0.0),
        "bein_triv": _affine_trivial(1.0, inputs["be_in"]),
        "lng0_triv": _affine_trivial(inputs["ln_g0"], 0.0),
        "lnb0_triv": _affine_trivial(1.0, inputs["ln_b0"]),
        "lng1_triv": _affine_trivial(inputs["ln_g1"], 0.0),
        "lnb1_triv": _affine_trivial(1.0, inputs["ln_b1"]),
        "tmb0_triv": _affine_trivial(1.0, inputs["tm_b0"]),
        "tmb1_triv": _affine_trivial(1.0, inputs["tm_b1"]),
        "bout_triv": _affine_trivial(1.0, inputs["b_out"]),
    }


def make_in_maps(inputs, prep, flags):
    import ml_dtypes
    bf16 = ml_dtypes.bfloat16

    def bc(v, width):
        return np.tile(np.asarray(v, np.float32).reshape(1, width), (128, 1))

    iota256 = np.tile(np.arange(256, dtype=np.float32)[None, :],
                      (128, 1)).astype(bf16)
    in_maps = []
    for k in range(NCORES):
        m = {
            "xT": prep["xT"][k].astype(bf16),
            "w_in": np.asarray(inputs["W_in"], np.float32).astype(bf16),
            "idx16": prep["idx16"][k],
            "rel": prep["rel"][k].astype(bf16),
            "inv": prep["inv"][k],
            "invfw": np.tile(prep["inv_fw"][k][None, :], (128, 1)),
            "tmw0": np.asarray(inputs["tm_W0"], np.float32).astype(bf16),
            "tmw1": np.asarray(inputs["tm_W1"], np.float32).astype(bf16),
            "wout": np.asarray(inputs["W_out"], np.float32).astype(bf16),
            "iota256": iota256,
            "ident": np.eye(128, dtype=np.float32),
        }
        if not flags["bin_triv"]:
            m["bin"] = bc(inputs["b_in"], F)
        if not flags["gin_triv"]:
            m["gin"] = bc(inputs["g_in"], F)
        if not flags["bein_triv"]:
            m["bein"] = bc(inputs["be_in"], F)
        for nm, src in (("lng0", "ln_g0"), ("lnb0", "ln_b0"),
                        ("lng1", "ln_g1"), ("lnb1", "ln_b1")):
            if not flags[nm + "_triv"]:
                m[nm] = bc(inputs[src], F)
        if not flags["tmb0_triv"]:
            m["tmb0"] = bc(inputs["tm_b0"], CH)
        if not flags["tmb1_triv"]:
            m["tmb1"] = bc(inputs["tm_b1"], CH)
        if not flags["bout_triv"]:
            m["bout"] = bc(inputs["b_out"], OUT_C)
        in_maps.append(m)
    return in_maps


def kernel(**inputs):
    return run(inputs, FULL_CFG)


# revision 7
# speedup vs baseline: 1.0763x; 1.0687x over previous
"""ONGNN (2-layer ordered-neuron GNN) on 8 Trainium2 NeuronCores — v2.

Same architecture as the baseline kernel (dst-node sharding, AllGather of
node features, indirect-DMA gather of source rows, one-hot-matmul segment
sum, node-parallel dense math), restructured to cut the SWDGE descriptor
load (the Q7 bottleneck) and overlap the collectives:

  - Edges are bucketed by (dst superwindow [512 nodes], src chunk-group)
    instead of (dst window [128], chunk): padding is only at bucket tails
    (pad slots gather row 0 and are masked by an out-of-range rel), cutting
    gather descriptors ~10% (150k -> 134.5k per core per layer).
  - Segment-sum matmuls keep the gathered message tile stationary and
    stream a one-hot over a 1- or 2-window dst range (bf16 iota compare is
    exact below 256), producing feature-major mT in PSUM: one matmul per
    message tile instead of one per (tile, window).  One-hot tiles are
    built in one batched is_equal per width class per bucket — keeping DVE
    2-port traffic low matters because DVE shares an SBUF port with the Q7
    cores that generate the gather descriptors (the critical path).
  - The node-feature table is AllGathered in 4 chunk-groups (pair-major row
    layout) so communication overlaps the dense phase that produces it and
    gathers start as soon as their chunk's group has arrived.
  - REPEAT iterations run inside one NEFF execution so steady-state
    throughput measurement is device-bound, not dispatch-bound.
"""
import sys
import numpy as np

sys.path.insert(0, "/opt/trn_rl_repo")

import concourse.bass as bass
import concourse.bacc as bacc
import concourse.mybir as mybir
import concourse.tile as tile
from concourse import bass_utils

F = 128       # feature dim (IN_C == HID)
CH = 64       # CHUNK
OUT_C = 40
EPS = 1e-5
NCORES = 8

SH = 12500        # dst nodes per core
WPC = 98          # 128-node windows per core
SW = 4            # windows per superwindow
SHP = WPC * 128   # padded shard rows (12544)
N_SW = (WPC + SW - 1) // SW          # 25 superwindows
N_PAIRS = (N_SW + 1) // 2            # 13 pairs
# chunk-groups: pairs [0,4), [4,8), [8,12), [12,13)
GROUP_PAIRS = [(0, 4), (4, 8), (8, 12), (12, 13)]
NG = len(GROUP_PAIRS)
GROW_CORE = [4096, 4096, 4096, 256]       # rows per core per group
GROWS8 = [g * NCORES for g in GROW_CORE]  # 32768, 32768, 32768, 2048
GBASE = [0, 32768, 65536, 98304]
PAD_REL = 1000.0

FULL_CFG = dict(N=100000, E=1000000)


def _host_prep(x, edge_index, cfg):
    """Bucket edges by (core, superwindow, group); build idx/rel streams and
    the shared matmul span schedule (union over cores per tile)."""
    src = np.asarray(edge_index[0], dtype=np.int64)
    dst = np.asarray(edge_index[1], dtype=np.int64)

    k_src = src // SH
    r_src = src - k_src * SH
    g_src = r_src // 4096
    row = (np.asarray(GBASE, np.int64)[g_src]
           + k_src * np.asarray(GROW_CORE, np.int64)[g_src]
           + (r_src - g_src * 4096))
    idx_loc = row - np.asarray(GBASE, np.int64)[g_src]   # < 32768

    core = dst // SH
    dloc = dst - core * SH
    win = dloc >> 7
    sw = win >> 2
    rel = (dloc - sw * 512).astype(np.float32)           # 0..511

    bucket = ((core * N_SW + sw) * NG + g_src).astype(np.int64)
    order = np.lexsort((idx_loc, win, bucket))
    bcnt = np.bincount(bucket, minlength=NCORES * N_SW * NG) \
        .reshape(NCORES, N_SW, NG)
    maxreal = bcnt.max(axis=0)                           # [N_SW, NG]
    nt = -(-maxreal // 128)                              # tiles per bucket

    starts = np.zeros(NCORES * N_SW * NG + 1, np.int64)
    np.cumsum(bcnt.reshape(-1), out=starts[1:])

    # tile offsets per bucket (shared schedule)
    t0 = np.zeros((N_SW, NG), np.int64)
    acc = 0
    for s in range(N_SW):
        for g in range(NG):
            t0[s, g] = acc
            acc += nt[s, g]
    T = int(acc)

    idx16 = np.zeros((NCORES, T * 128), np.int16)
    relst = np.full((NCORES, T * 128), PAD_REL, np.float32)
    # spans: for each (sw, g, tile): set of local windows present (any core)
    span_sets = {}
    for s in range(N_SW):
        for g in range(NG):
            for t in range(int(nt[s, g])):
                span_sets[(s, g, t)] = set()

    for k in range(NCORES):
        for s in range(N_SW):
            for g in range(NG):
                b = (k * N_SW + s) * NG + g
                sel = order[starts[b]:starts[b + 1]]
                n = sel.size
                mr = int(maxreal[s, g])
                if mr == 0:
                    continue
                pos = t0[s, g] * 128
                idx16[k, pos:pos + n] = idx_loc[sel].astype(np.int16)
                # pads keep idx 0: gathered (valid row) but masked by rel
                relst[k, pos:pos + n] = rel[sel]
                wl = (rel[sel].astype(np.int64)) >> 7    # local window 0..3
                tl = np.arange(n) // 128
                for t, w in zip(tl, wl):
                    span_sets[(s, g, int(t))].add(int(w))

    # one matmul per (tile, window range): mm = (t_local, wlo, width, dummy)
    # width-1 mms first within each run so one-hot builds batch per width
    sched = []          # per sw: list of (g, t0, nt, mms_run)
    M = 0
    for s in range(N_SW):
        w_in_sw = min(SW, WPC - s * SW)
        runs = []
        covered = set()
        for g in range(NG):
            if nt[s, g] == 0:
                continue
            mm1, mm2 = [], []
            for t in range(int(nt[s, g])):
                W = span_sets[(s, g, t)] or {0}
                wlo, whi = min(W), max(W)
                if whi == wlo:
                    mm1.append((t, wlo, 1, False))
                    covered.add(wlo)
                elif whi - wlo == 1:
                    mm2.append((t, wlo, 2, False))
                    covered.update((wlo, wlo + 1))
                else:
                    mm2.append((t, wlo, 2, False))
                    w2 = min(wlo + 2, w_in_sw - 2)
                    mm2.append((t, w2, 2, False))
                    covered.update(range(wlo, wlo + 2))
                    covered.update(range(w2, w2 + 2))
            runs.append([g, int(t0[s, g]), int(nt[s, g]), mm1 + mm2])
        assert runs, f"superwindow {s} has no tiles"
        missing = set(range(w_in_sw)) - covered
        for w in sorted(missing):
            runs[0][3].insert(0, (0, w, 1, True))
        sched.append([tuple(r) for r in runs])
        M += sum(len(r[3]) for r in runs)

    # relmm: per matmul, the 128 messages' dst offsets within the matmul's
    # window range ([0, width*128), bf16-exact), or OOB
    relmm = np.full((NCORES, M, 128), PAD_REL, np.float32)
    mm_i = 0
    for s in range(N_SW):
        for g, gt0, gnt, mms_run in sched[s]:
            for t, wlo, width, dummy in mms_run:
                if not dummy:
                    for k in range(NCORES):
                        r = relst[k, (gt0 + t) * 128:(gt0 + t + 1) * 128] \
                            - wlo * 128
                        relmm[k, mm_i] = np.where(
                            (r >= 0) & (r < width * 128), r, PAD_REL)
                mm_i += 1
    assert mm_i == M

    idx16_dev = np.ascontiguousarray(
        idx16.reshape(NCORES, T * 8, 16).transpose(0, 2, 1))   # [NC,16,T*8]
    idx16_dev = np.tile(idx16_dev, (1, 8, 1))                  # [NC,128,T*8]
    rel_dev = np.ascontiguousarray(relmm.transpose(0, 2, 1))   # [NC,128,M]

    cnt = np.bincount(dst, minlength=cfg["N"]).astype(np.float32)
    inv = 1.0 / np.maximum(cnt, 1.0)
    inv_dev = np.ones((NCORES, SHP), np.float32)
    for k in range(NCORES):
        inv_dev[k, :SH] = inv[k * SH:(k + 1) * SH]
    inv_dev = np.ascontiguousarray(
        inv_dev.reshape(NCORES, WPC, 128).transpose(0, 2, 1))

    xT_dev = np.zeros((NCORES, F, SHP), np.float32)
    x = np.asarray(x, np.float32)
    for k in range(NCORES):
        xT_dev[k, :, :SH] = x[k * SH:(k + 1) * SH].T

    inv_fw = np.ones((NCORES, SHP), np.float32)
    for k in range(NCORES):
        inv_fw[k, :SH] = inv[k * SH:(k + 1) * SH]

    return dict(idx16=idx16_dev, rel=rel_dev, inv=inv_dev, inv_fw=inv_fw,
                xT=xT_dev, sched=sched, T=T, M=M)


def _affine_trivial(g, b):
    return bool(np.allclose(g, 1.0, atol=1e-7) and np.allclose(b, 0.0, atol=1e-7))


def build_nc(cfg, prep, flags, repeat=1):
    AOp = mybir.AluOpType
    AF = mybir.ActivationFunctionType
    dt = mybir.dt

    T = prep["T"]
    M = prep["M"]
    sched = prep["sched"]

    nc = bacc.Bacc("TRN2", target_bir_lowering=False, debug=False,
                   num_devices=NCORES)

    # ---- I/O ----
    xT_d = nc.dram_tensor("xT", [F, SHP], dt.bfloat16, kind="ExternalInput")
    win_d = nc.dram_tensor("w_in", [F, F], dt.bfloat16, kind="ExternalInput")
    idx_d = nc.dram_tensor("idx16", [128, T * 8], dt.int16, kind="ExternalInput")
    rel_d = nc.dram_tensor("rel", [128, M], dt.bfloat16, kind="ExternalInput")
    inv_d = nc.dram_tensor("inv", [128, WPC], dt.float32, kind="ExternalInput")
    invfw_d = nc.dram_tensor("invfw", [128, SHP], dt.float32,
                             kind="ExternalInput")
    tmw0_d = nc.dram_tensor("tmw0", [2 * F, CH], dt.bfloat16, kind="ExternalInput")
    tmw1_d = nc.dram_tensor("tmw1", [2 * F, CH], dt.bfloat16, kind="ExternalInput")
    wout_d = nc.dram_tensor("wout", [F, OUT_C], dt.bfloat16, kind="ExternalInput")
    iota_d = nc.dram_tensor("iota256", [128, 256], dt.bfloat16, kind="ExternalInput")
    ident_d = nc.dram_tensor("ident", [128, 128], dt.float32, kind="ExternalInput")
    out_d = nc.dram_tensor("out", [SHP, OUT_C], dt.float32, kind="ExternalOutput")
    gb_d = {}
    for nm in ("bin", "gin", "bein", "lng0", "lnb0", "lng1", "lnb1", "tmb0",
               "tmb1", "bout"):
        if not flags[nm + "_triv"]:
            width = {"tmb0": CH, "tmb1": CH, "bout": OUT_C}.get(nm, F)
            gb_d[nm] = nc.dram_tensor(nm, [128, width], dt.float32,
                                      kind="ExternalInput")

    bounce = [[nc.dram_tensor(f"bounce{l}g{g}", [GROW_CORE[g], F], dt.bfloat16)
               for g in range(NG)] for l in range(2)]
    hg = [[nc.dram_tensor(f"hfull{l}g{g}", [GROWS8[g], F], dt.bfloat16,
                          addr_space="Shared")
           for g in range(NG)] for l in range(2)]

    with tile.TileContext(nc) as tc:
        import contextlib
        ctx = contextlib.ExitStack()
        with ctx:
            res = ctx.enter_context(tc.tile_pool(name="res", bufs=1))
            gpool = ctx.enter_context(tc.tile_pool(name="gpool", bufs=3))
            ohpool = ctx.enter_context(tc.tile_pool(name="ohpool", bufs=2))
            stream = ctx.enter_context(tc.tile_pool(name="stream", bufs=2))
            tiny = ctx.enter_context(tc.tile_pool(name="tiny", bufs=3))
            small = ctx.enter_context(tc.tile_pool(name="small", bufs=3))
            psum = ctx.enter_context(tc.tile_pool(name="psum", bufs=3, space="PSUM"))
            pst = ctx.enter_context(tc.tile_pool(name="pst", bufs=2, space="PSUM"))

            # ---- residents / constants ----
            hT = res.tile([128, SHP], dt.bfloat16, tag="hT")
            h_own = res.tile([128, SHP], dt.bfloat16, tag="h_own")
            sigc = res.tile([128, WPC * CH], dt.bfloat16, tag="sigc")
            rel_t = res.tile([128, M], dt.bfloat16, tag="rel")
            idx_t = res.tile([128, T * 8], dt.int16, tag="idx")
            inv_t = res.tile([128, WPC], dt.float32, tag="inv")
            w_in = res.tile([F, F], dt.bfloat16, tag="w_in")
            tmw = [[res.tile([F, CH], dt.bfloat16, tag=f"tmw{l}{h}", name=f"tmw{l}{h}")
                    for h in range(2)] for l in range(2)]
            wout = res.tile([F, OUT_C], dt.bfloat16, tag="wout")
            iota_t = res.tile([128, 256], dt.bfloat16, tag="iota256")
            ident = res.tile([128, 128], dt.float32, tag="ident")
            scanmask = res.tile([128, 2 * SW * CH], dt.float32, tag="scanmask")
            gb_t = {}
            for nm, d in gb_d.items():
                gb_t[nm] = res.tile(list(d.shape), dt.float32, tag=nm, name=nm)
                nc.sync.dma_start(gb_t[nm][:], d.ap())

            nc.sync.dma_start(rel_t[:], rel_d.ap())
            nc.sync.dma_start(idx_t[:], idx_d.ap())
            nc.sync.dma_start(inv_t[:], inv_d.ap())
            nc.sync.dma_start(w_in[:], win_d.ap())
            for l, d in enumerate((tmw0_d, tmw1_d)):
                nc.sync.dma_start(tmw[l][0][:], d.ap()[0:F, :])
                nc.sync.dma_start(tmw[l][1][:], d.ap()[F:2 * F, :])
            nc.sync.dma_start(wout[:], wout_d.ap())
            nc.sync.dma_start(iota_t[:], iota_d.ap())
            nc.sync.dma_start(ident[:], ident_d.ap())
            eps_t = res.tile([128, 1], dt.float32, tag="eps")
            nc.vector.memset(eps_t[:], EPS)
            nc.vector.memset(scanmask[:], 1.0)
            nc.vector.memset(
                scanmask[:].rearrange("p (w c) -> p w c", c=CH)[:, :, 0:1], 0.0)

            # gather-slot hygiene: pad columns (idx=-1, skipped) read stale
            # SBUF; zero the slots once so the very first reads are finite.
            ntmax = max((r[2] for runs in sched for r in runs), default=1)
            for _ in range(3):
                z = gpool.tile([128, ntmax * 128], dt.bfloat16, tag="g",
                               name="gz")
                nc.vector.memset(z[:], 0.0)

            PW = 2 * SW
            n_pairs = N_PAIRS

            def ln_smalls(su, sq, nw):
                mean = small.tile([128, nw], dt.float32, tag="mean")
                nc.vector.tensor_scalar(mean[:], su, 1.0 / F, None, AOp.mult)
                t1 = small.tile([128, nw], dt.float32, tag="t1")
                nc.vector.tensor_tensor(t1[:], mean[:], su, AOp.mult)
                t2 = small.tile([128, nw], dt.float32, tag="t2")
                nc.vector.tensor_tensor(t2[:], sq, t1[:], AOp.subtract)
                srt = small.tile([128, nw], dt.float32, tag="srt")
                nc.scalar.activation(srt[:], t2[:], AF.Sqrt, bias=eps_t[:],
                                     scale=1.0 / F)
                rs = small.tile([128, nw], dt.float32, tag="rs")
                nc.vector.reciprocal(rs[:], srt[:])
                return mean, rs

            def apply_ln(dst_ap, u_ap, mean, rs, nw, gnm, bnm):
                u3 = u_ap.rearrange("p (w f) -> p w f", w=nw)
                d3 = dst_ap.rearrange("p (w f) -> p w f", w=nw)
                mb = mean[:].unsqueeze(2).broadcast_to([128, nw, 128])
                rb = rs[:].unsqueeze(2).broadcast_to([128, nw, 128])
                nc.vector.tensor_tensor(d3, u3, mb, AOp.subtract)
                nc.vector.tensor_tensor(d3, d3, rb, AOp.mult)
                if gnm is not None:
                    g3 = gb_t[gnm][:].unsqueeze(1).broadcast_to([128, nw, 128])
                    nc.vector.tensor_tensor(d3, d3, g3, AOp.mult)
                if bnm is not None:
                    b3 = gb_t[bnm][:].unsqueeze(1).broadcast_to([128, nw, 128])
                    nc.vector.tensor_tensor(d3, d3, b3, AOp.add)

            def pair_info(pr):
                sws = [sx for sx in (2 * pr, 2 * pr + 1) if sx < N_SW]
                pw0 = sws[0] * SW
                nwp = sum(min(SW, WPC - sx * SW) for sx in sws)
                return sws, pw0, nwp

            def group_of_pair(pr):
                for g, (a, b) in enumerate(GROUP_PAIRS):
                    if a <= pr < b:
                        return g
                raise AssertionError

            def pair_bounce_rows(pr, g):
                a, _ = GROUP_PAIRS[g]
                r0 = (2 * pr - 2 * a) * SW * 128
                return r0

            def _once():
                # ============ Phase A: h0 = LN(relu(x W + b)) ============
                for pr in range(n_pairs):
                    sws, pw0, nwp = pair_info(pr)
                    nwfp = nwp * 128
                    xt8 = stream.tile([128, PW * 128], dt.bfloat16, tag="xt4",
                                      name="xt8")
                    nc.sync.dma_start(xt8[:, :nwfp],
                                      xT_d.ap()[:, pw0 * 128:pw0 * 128 + nwfp])
                    r8 = stream.tile([128, PW * 128], dt.float32, tag="u4", name="r8")
                    for sw in sws:
                        w0 = sw * SW
                        nw = min(SW, WPC - w0)
                        nwf = nw * 128
                        off = (w0 - pw0) * 128
                        ps_z = psum.tile([128, SW * 128], dt.float32, tag="ps_acc")
                        for j in range(nw):
                            nc.tensor.matmul(ps_z[:, j * 128:(j + 1) * 128],
                                             xt8[:, off + j * 128:off + (j + 1) * 128],
                                             w_in[:], start=True, stop=True)
                        if "bin" in gb_t:
                            b3 = gb_t["bin"][:].unsqueeze(1).broadcast_to(
                                [128, nw, 128])
                            z3 = ps_z[:, :nwf].rearrange("p (w f) -> p w f", w=nw)
                            nc.vector.tensor_tensor(z3, z3, b3, AOp.add)
                        nc.scalar.activation(r8[:, off:off + nwf], ps_z[:, :nwf],
                                             AF.Relu)
                    su = small.tile([128, PW], dt.float32, tag="su")
                    nc.vector.tensor_reduce(
                        su[:, :nwp], r8[:, :nwfp].rearrange("p (w f) -> p w f", w=nwp),
                        mybir.AxisListType.X, AOp.add)
                    sqs = stream.tile([128, PW * 128], dt.float32, tag="e4", name="sqs")
                    nc.scalar.activation(sqs[:, :nwfp], r8[:, :nwfp], AF.Square)
                    sq = small.tile([128, PW], dt.float32, tag="sq")
                    nc.vector.tensor_reduce(
                        sq[:, :nwp],
                        sqs[:, :nwfp].rearrange("p (w f) -> p w f", w=nwp),
                        mybir.AxisListType.X, AOp.add)
                    mean, rs = ln_smalls(su[:, :nwp], sq[:, :nwp], nwp)
                    apply_ln(h_own[:, pw0 * 128:pw0 * 128 + nwfp], r8[:, :nwfp],
                             mean, rs, nwp,
                             "gin" if "gin" in gb_t else None,
                             "bein" if "bein" in gb_t else None)
                    g = group_of_pair(pr)
                    r0 = pair_bounce_rows(pr, g)
                    nc.sync.dma_start(
                        bounce[0][g].ap()[r0:r0 + nwfp, :]
                            .rearrange("(w p) f -> p w f", w=nwp),
                        h_own[:, pw0 * 128:pw0 * 128 + nwfp]
                            .rearrange("p (w f) -> p w f", w=nwp))
                    nc.sync.dma_start_transpose(
                        hT[:, pw0 * 128:pw0 * 128 + nwfp],
                        bounce[0][g].ap()[r0:r0 + nwfp, :])
                    if pr == GROUP_PAIRS[g][1] - 1:
                        nc.gpsimd.collective_compute(
                            "AllGather", AOp.bypass,
                            replica_groups=[list(range(NCORES))],
                            ins=[bounce[0][g].ap().opt()],
                            outs=[hg[0][g].ap().opt()])

                # ============ conv layers ============
                for l in range(2):
                    mm_base = 0
                    mm_sw0 = []
                    for sw in range(N_SW):
                        mm_sw0.append(mm_base)
                        mm_base += sum(len(r[3]) for r in sched[sw])
                    for pr in range(n_pairs):
                        sws, pw0, nwp = pair_info(pr)
                        nwfp = nwp * 128
                        mTf = stream.tile([128, PW * 128], dt.float32, tag="m4")
                        mt8 = tiny.tile([128, PW * 128], dt.bfloat16, tag="mt")
                        for sw in sws:
                            w0 = sw * SW
                            nw = min(SW, WPC - w0)
                            nwf = nw * 128
                            off = (w0 - pw0) * 128
                            runs = sched[sw]
                            ps_m = psum.tile([128, SW * 128], dt.float32,
                                             tag="ps_acc")
                            mm_i = mm_sw0[sw]
                            first = True
                            for ri, (g, gt0, gnt, mms_run) in enumerate(runs):
                                g_t = gpool.tile([128, gnt * 128], dt.bfloat16,
                                                 tag="g")
                                nc.gpsimd.dma_gather(
                                    g_t[:].rearrange("p (t f) -> p t f", t=gnt),
                                    hg[l][g].ap(),
                                    idx_t[:, gt0 * 8:(gt0 + gnt) * 8],
                                    gnt * 128, gnt * 128, F,
                                    single_packet=False)
                                nmm_r = len(mms_run)
                                n1 = sum(1 for mm in mms_run if mm[2] == 1)
                                n2 = nmm_r - n1
                                oh_t = ohpool.tile(
                                    [128, n1 * 128 + n2 * 256],
                                    dt.bfloat16, tag="oh")
                                if n1:
                                    oh3 = oh_t[:, :n1 * 128].rearrange(
                                        "p (m c) -> p m c", m=n1)
                                    iob = iota_t[:, :128].unsqueeze(1) \
                                        .broadcast_to([128, n1, 128])
                                    reb = rel_t[:, mm_i:mm_i + n1] \
                                        .unsqueeze(2) \
                                        .broadcast_to([128, n1, 128])
                                    nc.vector.tensor_tensor(oh3, iob, reb,
                                                            AOp.is_equal)
                                if n2:
                                    oh3 = oh_t[:, n1 * 128:].rearrange(
                                        "p (m c) -> p m c", m=n2)
                                    iob = iota_t[:].unsqueeze(1) \
                                        .broadcast_to([128, n2, 256])
                                    reb = rel_t[:, mm_i + n1:mm_i + nmm_r] \
                                        .unsqueeze(2) \
                                        .broadcast_to([128, n2, 256])
                                    nc.vector.tensor_tensor(oh3, iob, reb,
                                                            AOp.is_equal)
                                oh_off = 0
                                for j, (tl, wlo, width, dummy) in \
                                        enumerate(mms_run):
                                    is_last = (ri == len(runs) - 1
                                               and j == nmm_r - 1)
                                    wf = width * 128
                                    nc.tensor.matmul(
                                        ps_m[:, wlo * 128:wlo * 128 + wf],
                                        g_t[:, tl * 128:(tl + 1) * 128],
                                        oh_t[:, oh_off:oh_off + wf],
                                        start=first, stop=is_last)
                                    first = False
                                    oh_off += wf
                                mm_i += nmm_r
                            # mT (f32) into the pair tile (feature-major);
                            # ACT copy keeps the DVE SBUF port free for SWDGE
                            nc.scalar.activation(mTf[:, off:off + nwf],
                                                 ps_m[:, :nwf], AF.Copy)
                        # m~T (bf16, inv-scaled) for the transition matmul
                        ivf = tiny.tile([128, PW * 128], dt.float32, tag="ivf")
                        nc.sync.dma_start(
                            ivf[:, :nwfp],
                            invfw_d.ap()[:, pw0 * 128:pw0 * 128 + nwfp])
                        nc.vector.tensor_tensor(mt8[:, :nwfp], mTf[:, :nwfp],
                                                ivf[:, :nwfp], AOp.mult)
                        # m node-major (f32, inv-scaled): transpose mT
                        m4 = stream.tile([128, PW * 128], dt.float32, tag="m4n")
                        for half in range(0, nwp, SW):
                            nh = min(SW, nwp - half)
                            ps_t = pst.tile([128, SW * 128], dt.float32, tag="ps_t")
                            for j in range(nh):
                                nc.tensor.transpose(
                                    ps_t[:, j * 128:(j + 1) * 128],
                                    mTf[:, (half + j) * 128:(half + j + 1) * 128],
                                    ident[:])
                            m3 = m4[:, half * 128:(half + nh) * 128] \
                                .rearrange("p (w f) -> p w f", w=nh)
                            p3 = ps_t[:, :nh * 128] \
                                .rearrange("p (w f) -> p w f", w=nh)
                            iv = inv_t[:, pw0 + half:pw0 + half + nh] \
                                .unsqueeze(2).broadcast_to([128, nh, 128])
                            nc.vector.tensor_tensor(m3, p3, iv, AOp.mult)
                        # ---- dense phase over the whole pair ----
                        ps_tm = psum.tile([128, PW * CH], dt.float32, tag="ps_sm")
                        for j in range(nwp):
                            w = pw0 + j
                            nc.tensor.matmul(ps_tm[:, j * CH:(j + 1) * CH],
                                             hT[:, w * 128:(w + 1) * 128],
                                             tmw[l][0][:], start=True, stop=False)
                            nc.tensor.matmul(ps_tm[:, j * CH:(j + 1) * CH],
                                             mt8[:, j * 128:(j + 1) * 128],
                                             tmw[l][1][:], start=False, stop=True)
                        nwc = nwp * CH
                        if ("tmb0", "tmb1")[l] in gb_t:
                            tb = gb_t[("tmb0", "tmb1")[l]][:].unsqueeze(1) \
                                .broadcast_to([128, nwp, CH])
                            z3 = ps_tm[:, :nwc].rearrange("p (w c) -> p w c", w=nwp)
                            nc.vector.tensor_tensor(z3, z3, tb, AOp.add)
                        # softmax (no max-sub) + cumsum
                        e4 = stream.tile([128, PW * CH], dt.float32, tag="e4")
                        nc.scalar.activation(e4[:, :nwc], ps_tm[:, :nwc], AF.Exp)
                        s4 = small.tile([128, PW], dt.float32, tag="s4")
                        nc.vector.tensor_reduce(
                            s4[:, :nwp],
                            e4[:, :nwc].rearrange("p (w c) -> p w c", w=nwp),
                            mybir.AxisListType.X, AOp.add)
                        r4s = small.tile([128, PW], dt.float32, tag="r4s")
                        nc.vector.reciprocal(r4s[:, :nwp], s4[:, :nwp])
                        cs4 = stream.tile([128, PW * CH], dt.float32, tag="cs4")
                        nc.vector.tensor_tensor_scan(
                            cs4[:, :nwc], scanmask[:, :nwc], e4[:, :nwc],
                            0.0, AOp.mult, AOp.add)
                        # sig update
                        rb = r4s[:, :nwp].unsqueeze(2).broadcast_to([128, nwp, CH])
                        cs3 = cs4[:, :nwc].rearrange("p (w c) -> p w c", w=nwp)
                        sg_cols = sigc[:, pw0 * CH:pw0 * CH + nwc]
                        sg3 = sg_cols.rearrange("p (w c) -> p w c", w=nwp)
                        if l == 0:
                            nc.vector.tensor_tensor(sg3, cs3, rb, AOp.mult)
                            sig_src = sg_cols
                        else:
                            t4 = stream.tile([128, PW * CH], dt.float32, tag="t4")
                            t3 = t4[:, :nwc].rearrange("p (w c) -> p w c", w=nwp)
                            nc.vector.tensor_tensor(t3, cs3, rb, AOp.mult)
                            a4 = stream.tile([128, PW * CH], dt.float32, tag="a4")
                            nc.vector.tensor_tensor(a4[:, :nwc], sg_cols,
                                                    t4[:, :nwc], AOp.mult)
                            nc.vector.tensor_tensor(t4[:, :nwc], t4[:, :nwc],
                                                    a4[:, :nwc], AOp.subtract)
                            nc.vector.tensor_tensor(t4[:, :nwc], t4[:, :nwc],
                                                    sg_cols, AOp.add)
                            sig_src = t4[:, :nwc]
                        # mix u = m + sig*(h-m)
                        hcols = h_own[:, pw0 * 128:pw0 * 128 + nwfp]
                        u4 = stream.tile([128, PW * 128], dt.float32, tag="u4")
                        nc.vector.tensor_tensor(u4[:, :nwfp], hcols, m4[:, :nwfp],
                                                AOp.subtract)
                        src_b = sig_src.rearrange("p (w c) -> p w c", w=nwp) \
                            .unsqueeze(3).broadcast_to([128, nwp, CH, 2])
                        u4v = u4[:, :nwfp].rearrange("p (w c r) -> p w c r",
                                                     w=nwp, r=2)
                        nc.vector.tensor_tensor(u4v, u4v, src_b, AOp.mult)
                        nc.vector.tensor_tensor(u4[:, :nwfp], u4[:, :nwfp],
                                                m4[:, :nwfp], AOp.add)
                        # LN stats
                        su = small.tile([128, PW], dt.float32, tag="su")
                        nc.vector.tensor_reduce(
                            su[:, :nwp],
                            u4[:, :nwfp].rearrange("p (w f) -> p w f", w=nwp),
                            mybir.AxisListType.X, AOp.add)
                        sqs = stream.tile([128, PW * 128], dt.float32, tag="e4",
                                          name="sqs")
                        nc.scalar.activation(sqs[:, :nwfp], u4[:, :nwfp], AF.Square)
                        sq = small.tile([128, PW], dt.float32, tag="sq")
                        nc.vector.tensor_reduce(
                            sq[:, :nwp],
                            sqs[:, :nwfp].rearrange("p (w f) -> p w f", w=nwp),
                            mybir.AxisListType.X, AOp.add)
                        mean, rs = ln_smalls(su[:, :nwp], sq[:, :nwp], nwp)
                        gnm = ("lng0", "lng1")[l]
                        bnm = ("lnb0", "lnb1")[l]
                        if l == 0:
                            apply_ln(hcols, u4[:, :nwfp], mean, rs, nwp,
                                     gnm if gnm in gb_t else None,
                                     bnm if bnm in gb_t else None)
                            g = group_of_pair(pr)
                            r0 = pair_bounce_rows(pr, g)
                            nc.sync.dma_start(
                                bounce[1][g].ap()[r0:r0 + nwfp, :]
                                    .rearrange("(w p) f -> p w f", w=nwp),
                                hcols.rearrange("p (w f) -> p w f", w=nwp))
                            nc.sync.dma_start_transpose(
                                hT[:, pw0 * 128:pw0 * 128 + nwfp],
                                bounce[1][g].ap()[r0:r0 + nwfp, :])
                            if pr == GROUP_PAIRS[g][1] - 1:
                                nc.gpsimd.collective_compute(
                                    "AllGather", AOp.bypass,
                                    replica_groups=[list(range(NCORES))],
                                    ins=[bounce[1][g].ap().opt()],
                                    outs=[hg[1][g].ap().opt()])
                        else:
                            h2 = stream.tile([128, PW * 128], dt.float32, tag="hx",
                                             name="h2")
                            apply_ln(h2[:, :nwfp], u4[:, :nwfp], mean, rs, nwp,
                                     gnm if gnm in gb_t else None,
                                     bnm if bnm in gb_t else None)
                            ob = stream.tile([128, PW * OUT_C], dt.float32,
                                             tag="ob")
                            ps_o = psum.tile([128, PW * OUT_C], dt.float32,
                                             tag="ps_sm")
                            h2t8 = tiny.tile([128, PW * 128], dt.bfloat16,
                                             tag="h2t")
                            for half in range(0, nwp, SW):
                                nh = min(SW, nwp - half)
                                ps_t = pst.tile([128, SW * 128], dt.float32,
                                                tag="ps_t")
                                for j in range(nh):
                                    nc.tensor.transpose(
                                        ps_t[:, j * 128:(j + 1) * 128],
                                        h2[:, (half + j) * 128:(half + j + 1) * 128],
                                        ident[:])
                                nc.scalar.activation(
                                    h2t8[:, half * 128:(half + nh) * 128],
                                    ps_t[:, :nh * 128], AF.Copy)
                            for j in range(nwp):
                                nc.tensor.matmul(ps_o[:, j * OUT_C:(j + 1) * OUT_C],
                                                 h2t8[:, j * 128:(j + 1) * 128],
                                                 wout[:], start=True, stop=True)
                            nwo = nwp * OUT_C
                            if "bout" in gb_t:
                                bb = gb_t["bout"][:].unsqueeze(1).broadcast_to(
                                    [128, nwp, OUT_C])
                                o3 = ob[:, :nwo].rearrange("p (w o) -> p w o", w=nwp)
                                nc.vector.tensor_tensor(
                                    o3,
                                    ps_o[:, :nwo].rearrange("p (w o) -> p w o",
                                                            w=nwp),
                                    bb, AOp.add)
                            else:
                                nc.vector.tensor_copy(ob[:, :nwo], ps_o[:, :nwo])
                            nc.sync.dma_start(
                                out_d.ap()[pw0 * 128:pw0 * 128 + nwfp, :]
                                    .rearrange("(w p) o -> p w o", w=nwp),
                                ob[:, :nwo].rearrange("p (w o) -> p w o", w=nwp))

            for _rep in range(repeat):
                _once()

    nc.compile()
    return nc


_CACHE = {}


def _sched_key(prep):
    return tuple((g, t0, nt, tuple(mms))
                 for runs in prep["sched"] for g, t0, nt, mms in runs)


def _get_compiled(cfg, prep, flags, repeat=1):
    key = (_sched_key(prep), tuple(sorted(flags.items())), repeat)
    if key not in _CACHE:
        _CACHE[key] = build_nc(cfg, prep, flags, repeat=repeat)
    return _CACHE[key]


class PjrtRunner:
    """Persistent jitted shard_map executor for one compiled nc (8 cores)."""

    def __init__(self, nc):
        import jax
        from jax.experimental.shard_map import shard_map
        from jax.sharding import Mesh, PartitionSpec
        from concourse import bass2jax

        bass2jax.install_neuronx_cc_hook()
        self.nc = nc
        in_names, out_names, out_avals, zero_outs = [], [], [], []
        partition_name = (nc.partition_id_tensor.name
                          if nc.partition_id_tensor else None)
        for alloc in nc.m.functions[0].allocations:
            if not isinstance(alloc, mybir.MemoryLocationSet):
                continue
            name = alloc.memorylocations[0].name
            if alloc.kind == "ExternalInput":
                if name != partition_name:
                    in_names.append(name)
            elif alloc.kind == "ExternalOutput":
                out_names.append(name)
                aval = jax.core.ShapedArray(
                    tuple(alloc.tensor_shape), mybir.dt.np(alloc.dtype))
                out_avals.append(aval)
                zero_outs.append(np.zeros(alloc.tensor_shape,
                                          mybir.dt.np(alloc.dtype)))
        self.n_params = len(in_names)
        self.out_names = list(out_names)
        self.zero_outs = zero_outs
        all_in = in_names + out_names
        if partition_name is not None:
            all_in.append(partition_name)
        self.in_names_data = in_names

        def _body(*args):
            operands = list(args)
            if partition_name is not None:
                operands.append(bass2jax.partition_id_tensor())
            outs = bass2jax._bass_exec_p.bind(
                *operands,
                out_avals=tuple(out_avals),
                in_names=tuple(all_in),
                out_names=tuple(out_names),
                lowering_input_output_aliases=(),
                sim_require_finite=True,
                sim_require_nnan=True,
                nc=nc,
            )
            return tuple(outs)

        devices = jax.devices()[:NCORES]
        self.mesh = Mesh(np.asarray(devices), ("core",))
        n_out = len(out_names)
        donate = tuple(range(self.n_params, self.n_params + n_out))
        in_specs = (PartitionSpec("core"),) * (self.n_params + n_out)
        out_specs = (PartitionSpec("core"),) * n_out
        self.fn = jax.jit(
            shard_map(_body, mesh=self.mesh, in_specs=in_specs,
                      out_specs=out_specs, check_rep=False),
            donate_argnums=donate, keep_unused=True)

    def concat_inputs(self, in_maps):
        return [
            np.concatenate([np.asarray(in_maps[c][nm]) for c in range(NCORES)],
                           axis=0)
            for nm in self.in_names_data
        ]

    def zeros(self):
        return [np.zeros((NCORES * z.shape[0], *z.shape[1:]), z.dtype)
                for z in self.zero_outs]

    def __call__(self, concat_in, zeros):
        outs = self.fn(*concat_in, *zeros)
        return {nm: np.asarray(outs[i]) for i, nm in enumerate(self.out_names)}


_RUNNERS = {}


def get_runner(cfg, prep, flags, repeat=1):
    key = (_sched_key(prep), tuple(sorted(flags.items())), repeat)
    if key not in _RUNNERS:
        _RUNNERS[key] = PjrtRunner(_get_compiled(cfg, prep, flags, repeat=repeat))
    return _RUNNERS[key]


# Iterations per NEFF execution.  The axon client dispatch pipeline tops out
# at ~4 ms per execution; several iterations per execution make each dispatch
# device-bound so measured throughput reflects hardware time.
REPEAT = 4


def run(inputs, cfg):
    x = np.asarray(inputs["x"], np.float32)
    prep = _host_prep(x, np.asarray(inputs["edge_index"]), cfg)

    flags = make_flags(inputs)
    runner = get_runner(cfg, prep, flags, repeat=REPEAT)
    in_maps = make_in_maps(inputs, prep, flags)
    out = runner(runner.concat_inputs(in_maps), runner.zeros())["out"]
    out = out.reshape(NCORES, SHP, OUT_C)[:, :SH, :]
    return np.ascontiguousarray(out.reshape(NCORES * SH, OUT_C), dtype=np.float32)


def make_flags(inputs):
    return {
        "bin_triv": _affine_trivial(1.0, inputs["b_in"]),
        "gin_triv": _affine_trivial(inputs["g_in"], 0.0),
        "bein_triv": _affine_trivial(1.0, inputs["be_in"]),
        "lng0_triv": _affine_trivial(inputs["ln_g0"], 0.0),
        "lnb0_triv": _affine_trivial(1.0, inputs["ln_b0"]),
        "lng1_triv": _affine_trivial(inputs["ln_g1"], 0.0),
        "lnb1_triv": _affine_trivial(1.0, inputs["ln_b1"]),
        "tmb0_triv": _affine_trivial(1.0, inputs["tm_b0"]),
        "tmb1_triv": _affine_trivial(1.0, inputs["tm_b1"]),
        "bout_triv": _affine_trivial(1.0, inputs["b_out"]),
    }


def make_in_maps(inputs, prep, flags):
    import ml_dtypes
    bf16 = ml_dtypes.bfloat16

    def bc(v, width):
        return np.tile(np.asarray(v, np.float32).reshape(1, width), (128, 1))

    iota256 = np.tile(np.arange(256, dtype=np.float32)[None, :],
                      (128, 1)).astype(bf16)
    in_maps = []
    for k in range(NCORES):
        m = {
            "xT": prep["xT"][k].astype(bf16),
            "w_in": np.asarray(inputs["W_in"], np.float32).astype(bf16),
            "idx16": prep["idx16"][k],
            "rel": prep["rel"][k].astype(bf16),
            "inv": prep["inv"][k],
            "invfw": np.tile(prep["inv_fw"][k][None, :], (128, 1)),
            "tmw0": np.asarray(inputs["tm_W0"], np.float32).astype(bf16),
            "tmw1": np.asarray(inputs["tm_W1"], np.float32).astype(bf16),
            "wout": np.asarray(inputs["W_out"], np.float32).astype(bf16),
            "iota256": iota256,
            "ident": np.eye(128, dtype=np.float32),
        }
        if not flags["bin_triv"]:
            m["bin"] = bc(inputs["b_in"], F)
        if not flags["gin_triv"]:
            m["gin"] = bc(inputs["g_in"], F)
        if not flags["bein_triv"]:
            m["bein"] = bc(inputs["be_in"], F)
        for nm, src in (("lng0", "ln_g0"), ("lnb0", "ln_b0"),
                        ("lng1", "ln_g1"), ("lnb1", "ln_b1")):
            if not flags[nm + "_triv"]:
                m[nm] = bc(inputs[src], F)
        if not flags["tmb0_triv"]:
            m["tmb0"] = bc(inputs["tm_b0"], CH)
        if not flags["tmb1_triv"]:
            m["tmb1"] = bc(inputs["tm_b1"], CH)
        if not flags["bout_triv"]:
            m["bout"] = bc(inputs["b_out"], OUT_C)
        in_maps.append(m)
    return in_maps


def kernel(**inputs):
    return run(inputs, FULL_CFG)
